# revision 11
# baseline (speedup 1.0000x reference)
"""Trainium2 Bass kernel for nn_GAT_GCN (GAT -> GCN -> readout -> MLP), 8-core SPMD.

v2: inverted GCN (transform-then-aggregate) so the big AllGather pipelines
under the 6800x6800 GCN matmul instead of sitting exposed on the critical
path.

Sharding: 1024 rows per core: 992 graph-aligned nodes (32 graphs x 31) plus
32 orphan rows (the 64 readout-dropped nodes 7936..7999 split across cores
6 and 7). Edges are owned by their dst node; 8 dst blocks of 128 per core.

Pipeline per core:
- A: al = x @ [As|Ad] for OWN nodes only; AllGather -> alsp_pad[8192,32].
- B: GAT per dst block (one-hot mask matmul aggregation, softmax 1/z folded
  into the PSUM-drain activation scale); h written to DRAM kc-slab-major.
- T: h -> hT[128,54,1024] (feat-major, SBUF resident) via DMA transposes.
- E: xw = h @ W_gcn on own nodes (fp-chunk loop, wgcn streamed); xw rows
  re-transposed to node-major and AllGathered in 6 column segments, each
  issued as soon as its 9 fp chunks finish -> comm hides under E compute.
- D: GCN aggregation per segment via norm-weighted one-hot masks (gathering
  xw rows per edge), fused relu + graph max/mean readout; interleaved with
  E so segment s aggregates while segment s+1 transforms.
- MLP on [64,32] transposed output; host concatenates.
"""
import sys
import numpy as np
import ml_dtypes

sys.path.insert(0, "/opt/trn_rl_repo")

from contextlib import ExitStack  # noqa: E402

import concourse.bass as bass  # noqa: E402
import concourse.tile as tile  # noqa: E402
from concourse import bacc, mybir  # noqa: E402

N, E, G = 8000, 32000, 256
F, H = 680, 10
HF = F * H                    # 6800
NC_ = 8                       # cores
NPC = 992                     # readout nodes per core (32 graphs x 31)
RPC = 1024                    # rows per core (992 + 32 orphan slots)
NBLK = 8                      # dst blocks per core (128 each)
EB = 768                      # padded edges per block
EC = EB // 128                # 6 edge chunks
FP = 768                      # padded F
KC1 = FP // 128               # 6
HFP = 6912                    # padded HF
KC2 = HFP // 128              # 54
NSEG = 6                      # xw AllGather segments
SEGF = KC2 // NSEG            # 9 fp chunks per segment
SEGW = SEGF * 128             # 1152 cols per segment
GPC = 32                      # graphs per core
NPG = 31                      # nodes per graph

f32 = mybir.dt.float32
bf16 = mybir.dt.bfloat16
i32 = mybir.dt.int32
bfnp = ml_dtypes.bfloat16


# ----------------------------------------------------------------------------
# Host-side prep: sharding, padding, weight tiling
# ----------------------------------------------------------------------------

def node_owner_local(node):
    """node -> (owner core, local row) for the 992x8 + 32/32 orphan layout."""
    node = np.asarray(node)
    owner = np.where(node < 7936, node // NPC,
                     np.where(node < 7968, 6, 7))
    local = np.where(node < 7936, node - NPC * (node // NPC),
                     np.where(node < 7968, NPC + node - 7936,
                              NPC + node - 7968))
    return owner, local


def host_prep(inputs):
    x = np.asarray(inputs["x"], np.float32)
    edge_index = np.asarray(inputs["edge_index"])
    W_gat = np.asarray(inputs["W_gat"], np.float32)
    a_src = np.asarray(inputs["a_src"], np.float32)
    a_dst = np.asarray(inputs["a_dst"], np.float32)
    W_gcn = np.asarray(inputs["W_gcn"], np.float32)
    W1 = np.asarray(inputs["W1"], np.float32)
    W2 = np.asarray(inputs["W2"], np.float32)
    W3 = np.asarray(inputs["W3"], np.float32)
    for bname in ("b_gat", "b_gcn", "b1", "b2", "b3"):
        assert np.all(np.asarray(inputs[bname]) == 0), f"nonzero {bname}"

    src = np.concatenate([edge_index[0], np.arange(N)]).astype(np.int64)
    dst = np.concatenate([edge_index[1], np.arange(N)]).astype(np.int64)
    deg = np.bincount(dst, minlength=N).astype(np.float64)
    dinv = 1.0 / np.sqrt(deg)
    norm = (dinv[src] * dinv[dst]).astype(np.float32)

    owner_n, local_n = node_owner_local(np.arange(N))
    hpos = RPC * owner_n + local_n                       # node -> global row

    xb = np.zeros((N, FP), bfnp)
    xb[:, :F] = x.astype(bfnp)

    As = np.stack([W_gat[:, h * F:(h + 1) * F] @ a_src[h] for h in range(H)], 1)
    Ad = np.stack([W_gat[:, h * F:(h + 1) * F] @ a_dst[h] for h in range(H)], 1)
    ascat = np.zeros((FP, 64), bfnp)
    ascat[:F, :H] = As.astype(bfnp)
    ascat[:F, H:2 * H] = Ad.astype(bfnp)

    wgat = np.zeros((KC1, 128, HF), bfnp)
    wgat.reshape(FP, HF)[:F] = W_gat.astype(bfnp)

    wpad = np.zeros((HFP, HFP), np.float32)
    wpad[:HF, :HF] = W_gcn
    # [fp, i(k row in chunk), kc, j] -> per-partition contiguous DMA slabs
    wgcn = np.ascontiguousarray(
        wpad.reshape(KC2, 128, KC2, 128).transpose(2, 1, 0, 3)).astype(bfnp)

    # gT k-order: 54 gmp chunks (rows [0,HF) + 16 pad) then 54 gap chunks
    w1t = np.zeros((2 * KC2, 128, 512), bfnp)
    w1t.reshape(2 * HFP, 512)[:HF] = W1[:HF].astype(bfnp)
    w1t.reshape(2 * HFP, 512)[HFP:HFP + HF] = W1[HF:].astype(bfnp)
    w2t = np.ascontiguousarray(W2.reshape(4, 128, 128)).astype(bfnp)
    w3t = np.ascontiguousarray(W3).astype(bfnp)              # [128, 64]

    shared = dict(xb=xb, ascat=ascat, wgat=wgat, wgcn=wgcn,
                  w1t=w1t, w2t=w2t, w3t=w3t)

    # per-core own-node x slices (hpos-local row order)
    own_nodes = [np.where(owner_n == c)[0][np.argsort(local_n[owner_n == c])]
                 for c in range(NC_)]

    per_core = []
    for c in range(NC_):
        nodes_c = own_nodes[c]
        xown = np.zeros((RPC, FP), bfnp)
        xown[local_n[nodes_c]] = xb[nodes_c]

        esrcx = np.zeros((NBLK, EC, 128), np.int32)
        esrch = np.zeros((NBLK, EC, 128), np.int32)
        eldst = np.zeros((NBLK, EC, 128), np.int32)
        dlocc = np.full((NBLK, EC, 128), -1.0, np.float32)
        normc = np.zeros((NBLK, EC, 128), np.float32)
        em = (owner_n[dst] == c)
        es, ed, en = src[em], dst[em], norm[em]
        loc = local_n[ed]
        for b in range(NBLK):
            bm = (loc >= 128 * b) & (loc < 128 * (b + 1))
            cnt = int(bm.sum())
            assert cnt <= EB, (c, b, cnt)
            fs = np.zeros(EB, np.int64)
            fd = np.zeros(EB, np.int64)
            fl = np.full(EB, -1.0, np.float32)
            fn = np.zeros(EB, np.float32)
            fs[:cnt] = es[bm]
            fd[:cnt] = ed[bm]
            fl[:cnt] = (loc[bm] - 128 * b).astype(np.float32)
            fn[:cnt] = en[bm]
            esrcx[b] = fs.reshape(EC, 128)
            esrch[b] = hpos[fs].reshape(EC, 128)
            eldst[b] = hpos[fd].reshape(EC, 128)
            dlocc[b] = fl.reshape(EC, 128)
            normc[b] = fn.reshape(EC, 128)
        pc = dict(
            xown=xown,
            esrcx=np.ascontiguousarray(esrcx.reshape(NBLK * EC, 128).T),
            esrch=np.ascontiguousarray(esrch.reshape(NBLK * EC, 128).T),
            eldst=np.ascontiguousarray(eldst.reshape(NBLK * EC, 128).T),
            dlocc=np.ascontiguousarray(dlocc.reshape(NBLK * EC, 128).T),
            normc=np.ascontiguousarray(
                normc.reshape(NBLK * EC, 128).T.astype(bfnp)),
        )
        per_core.append(pc)
    return shared, per_core


# ----------------------------------------------------------------------------
# Device program (one SPMD Bass program; all per-core variation is via data)
# ----------------------------------------------------------------------------

def build_nc():
    nc = bacc.Bacc("TRN2", target_bir_lowering=False, debug=False,
                   num_devices=NC_)
    xb = nc.dram_tensor("xb", [N, FP], bf16, kind="ExternalInput").ap()
    xown = nc.dram_tensor("xown", [RPC, FP], bf16, kind="ExternalInput").ap()
    ascat = nc.dram_tensor("ascat", [FP, 64], bf16, kind="ExternalInput").ap()
    wgat = nc.dram_tensor("wgat", [KC1, 128, HF], bf16, kind="ExternalInput").ap()
    wgcn = nc.dram_tensor("wgcn", [KC2, 128, KC2, 128], bf16,
                          kind="ExternalInput").ap()
    w1t = nc.dram_tensor("w1t", [2 * KC2, 128, 512], bf16,
                         kind="ExternalInput").ap()
    w2t = nc.dram_tensor("w2t", [4, 128, 128], bf16, kind="ExternalInput").ap()
    w3t = nc.dram_tensor("w3t", [128, 64], bf16, kind="ExternalInput").ap()
    esrcx = nc.dram_tensor("esrcx", [128, NBLK * EC], i32,
                           kind="ExternalInput").ap()
    esrch = nc.dram_tensor("esrch", [128, NBLK * EC], i32,
                           kind="ExternalInput").ap()
    eldst = nc.dram_tensor("eldst", [128, NBLK * EC], i32,
                           kind="ExternalInput").ap()
    dlocc = nc.dram_tensor("dlocc", [128, NBLK * EC], f32,
                           kind="ExternalInput").ap()
    normc = nc.dram_tensor("normc", [128, NBLK * EC], bf16,
                           kind="ExternalInput").ap()
    outg = nc.dram_tensor("outg", [64, 32], f32, kind="ExternalOutput").ap()

    with tile.TileContext(nc) as tc, ExitStack() as ctx:
        dram = ctx.enter_context(tc.tile_pool(name="dram", bufs=1, space="DRAM"))
        alsl = dram.tile([RPC, 32], f32, name="alsl")
        alsp = dram.tile([NC_ * RPC, 32], f32, name="alsp", addr_space="Shared")
        hsl = dram.tile([KC2, RPC, 128], bf16, name="hsl")
        xwsl = [dram.tile([RPC, SEGW], bf16, name=f"xwsl{s}")
                for s in range(NSEG)]
        xwpad = [dram.tile([NC_ * RPC, SEGW], bf16, name=f"xwpad{s}",
                           addr_space="Shared") for s in range(NSEG)]
        singles = ctx.enter_context(tc.tile_pool(name="singles", bufs=1))

        iota_i = singles.tile([128, 128], i32)
        nc.gpsimd.iota(iota_i, pattern=[[1, 128]], base=0, channel_multiplier=0)
        iota_f = singles.tile([128, 128], f32)
        nc.vector.tensor_copy(iota_f, iota_i)

        ascat_sb = singles.tile([128, KC1, 64], bf16)
        nc.sync.dma_start(out=ascat_sb,
                          in_=ascat.rearrange("(c p) d -> p c d", p=128))
        esrcx_sb = singles.tile([128, NBLK * EC], i32)
        nc.sync.dma_start(out=esrcx_sb, in_=esrcx)
        esrch_sb = singles.tile([128, NBLK * EC], i32)
        nc.sync.dma_start(out=esrch_sb, in_=esrch)
        eldst_sb = singles.tile([128, NBLK * EC], i32)
        nc.sync.dma_start(out=eldst_sb, in_=eldst)
        dlocc_sb = singles.tile([128, NBLK * EC], f32)
        nc.sync.dma_start(out=dlocc_sb, in_=dlocc)
        normc_sb = singles.tile([128, NBLK * EC], bf16)
        nc.sync.dma_start(out=normc_sb, in_=normc)

        from concourse.masks import make_identity
        ident = singles.tile([128, 128], bf16, name="ident")
        make_identity(nc, ident)

        # ---------------- Phase A: al = x_own @ [As|Ad] -> AllGather ----------
        with tc.tile_pool(name="pa_sb", bufs=3) as pool, \
             tc.tile_pool(name="pa_ps", bufs=2, space="PSUM") as pps, \
             tc.tile_pool(name="pa_pt", bufs=4, space="PSUM") as ppt:
            for i in range(RPC // 128):
                r0 = 128 * i
                xr = pool.tile([128, FP], bf16, tag="xr")
                nc.sync.dma_start(out=xr, in_=xown[r0:r0 + 128, :])
                xt = pool.tile([128, KC1, 128], bf16, tag="xt")
                for k in range(KC1):
                    pt = ppt.tile([128, 128], bf16, tag="pt")
                    nc.tensor.transpose(
                        out=pt, in_=xr[:, 128 * k:128 * (k + 1)],
                        identity=ident)
                    nc.vector.tensor_copy(xt[:, k, :], pt)
                pal = pps.tile([128, 2 * H], f32, tag="pal")
                for k in range(KC1):
                    nc.tensor.matmul(pal, xt[:, k, :],
                                     ascat_sb[:, k, :2 * H],
                                     start=(k == 0), stop=(k == KC1 - 1))
                al_sb = pool.tile([128, 32], f32, tag="al")
                nc.vector.tensor_copy(al_sb[:, :2 * H], pal)
                nc.vector.memset(al_sb[:, 2 * H:], 0.0)
                nc.sync.dma_start(out=alsl[r0:r0 + 128, :], in_=al_sb)
        nc.gpsimd.collective_compute(
            "AllGather", mybir.AluOpType.bypass,
            replica_groups=[list(range(NC_))],
            ins=[alsl], outs=[alsp])

        # ---------------- Phase B: GAT blocks -> hsl (kc-slab-major) ---------
        with tc.tile_pool(name="pb_w", bufs=1) as pw, \
             tc.tile_pool(name="pb_sb", bufs=2) as pool, \
             tc.tile_pool(name="pb_sm", bufs=3) as psm, \
             tc.tile_pool(name="pb_m", bufs=EC + 2) as pm, \
             tc.tile_pool(name="pb_ps", bufs=2, space="PSUM") as pps, \
             tc.tile_pool(name="pb_ph", bufs=1, space="PSUM") as pph, \
             tc.tile_pool(name="pb_pz", bufs=2, space="PSUM") as ppz:
            wgat_sb = pw.tile([128, KC1, HF], bf16)
            for k in range(KC1):
                nc.sync.dma_start(out=wgat_sb[:, k, :], in_=wgat[k])

            for b in range(NBLK):
                xe = pool.tile([128, EC, FP], bf16, tag="xe")
                als = pool.tile([128, EC, 32], f32, tag="als")
                ald = pool.tile([128, EC, 32], f32, tag="ald")
                for e in range(EC):
                    col = b * EC + e
                    nc.gpsimd.indirect_dma_start(
                        out=xe[:, e, :], out_offset=None, in_=xb,
                        in_offset=bass.IndirectOffsetOnAxis(
                            ap=esrcx_sb[:, col:col + 1], axis=0))
                    nc.gpsimd.indirect_dma_start(
                        out=als[:, e, :], out_offset=None, in_=alsp,
                        in_offset=bass.IndirectOffsetOnAxis(
                            ap=esrch_sb[:, col:col + 1], axis=0))
                    nc.gpsimd.indirect_dma_start(
                        out=ald[:, e, :], out_offset=None, in_=alsp,
                        in_offset=bass.IndirectOffsetOnAxis(
                            ap=eldst_sb[:, col:col + 1], axis=0))

                masks = []
                exb = psm.tile([128, EC, H], bf16, tag="exb")
                for e in range(EC):
                    col = b * EC + e
                    msk = pm.tile([128, 128], bf16, tag="msk")
                    nc.vector.tensor_tensor(
                        out=msk,
                        in0=dlocc_sb[:, col:col + 1].to_broadcast([128, 128]),
                        in1=iota_f, op=mybir.AluOpType.is_equal)
                    masks.append(msk)
                    # logits -> exp (leaky_relu slope 0.2)
                    lg = psm.tile([128, H], f32, tag="lg")
                    nc.vector.tensor_tensor(out=lg, in0=als[:, e, :H],
                                            in1=ald[:, e, H:2 * H],
                                            op=mybir.AluOpType.add)
                    lg2 = psm.tile([128, H], f32, tag="lg2")
                    nc.vector.tensor_scalar_mul(lg2, lg, 0.2)
                    nc.vector.tensor_tensor(out=lg, in0=lg, in1=lg2,
                                            op=mybir.AluOpType.max)
                    nc.scalar.activation(out=exb[:, e, :], in_=lg,
                                         func=mybir.ActivationFunctionType.Exp)

                # z[d,h] = sum_e mask[e,d] * ex[e,h]
                pz = ppz.tile([128, H], f32, tag="pz")
                for e in range(EC):
                    nc.tensor.matmul(pz, masks[e], exb[:, e, :],
                                     start=(e == 0), stop=(e == EC - 1))
                zf = psm.tile([128, H], f32, tag="zf")
                nc.scalar.activation(out=zf, in_=pz,
                                     func=mybir.ActivationFunctionType.Copy,
                                     bias=1e-30)
                zinv = psm.tile([128, H], f32, tag="zinv")
                nc.vector.reciprocal(zinv, zf)

                # M_e[:, h, :] = mask_e * ex[e, h]  (vector/scalar split)
                aggT = pool.tile([128, KC1, H, 128], bf16, tag="aggT")
                Ms = []
                for e in range(EC):
                    Me = pm.tile([128, H, 128], bf16, tag="Me")
                    for h in range(H):
                        nc.vector.tensor_tensor(
                            out=Me[:, h, :], in0=masks[e],
                            in1=exb[:, e, h:h + 1].to_broadcast([128, 128]),
                            op=mybir.AluOpType.mult)
                    Ms.append(Me)
                # aggT[f, (h d)] += xe.T @ M
                for k in range(KC1):
                    for half in range(2):
                        pa = pps.tile([128, 5 * 128], f32, tag="pa")
                        h0 = 5 * half
                        for e in range(EC):
                            lhs = xe[:, e, 128 * k:128 * (k + 1)]
                            nc.tensor.matmul(pa[:, 0:512], lhs,
                                             Ms[e][:, h0:h0 + 4, :],
                                             start=(e == 0), stop=(e == EC - 1))
                            nc.tensor.matmul(pa[:, 512:640], lhs,
                                             Ms[e][:, h0 + 4:h0 + 5, :],
                                             start=(e == 0), stop=(e == EC - 1))
                        nc.vector.tensor_copy(aggT[:, k, h0:h0 + 5, :], pa)

                # transform per head + fused 1/z scale + relu
                h1 = pool.tile([128, HFP], bf16, tag="h1")
                for h in range(H):
                    ph = pph.tile([128, F], f32, tag="ph")
                    for k in range(KC1):
                        lhs = aggT[:, k, h, :]
                        nc.tensor.matmul(ph[:, 0:512], lhs,
                                         wgat_sb[:, k, F * h:F * h + 512],
                                         start=(k == 0), stop=(k == KC1 - 1))
                        nc.tensor.matmul(ph[:, 512:F], lhs,
                                         wgat_sb[:, k, F * h + 512:F * (h + 1)],
                                         start=(k == 0), stop=(k == KC1 - 1))
                    nc.scalar.activation(out=h1[:, F * h:F * (h + 1)], in_=ph,
                                         func=mybir.ActivationFunctionType.Relu,
                                         scale=zinv[:, h:h + 1])
                nc.vector.memset(h1[:, HF:HFP], 0.0)
                for kc in range(KC2):
                    nc.sync.dma_start(
                        out=hsl[kc, 128 * b:128 * (b + 1), :],
                        in_=h1[:, 128 * kc:128 * (kc + 1)])

        # ---------------- big resident tiles for T/E/D ----------------
        with tc.tile_pool(name="pe_big", bufs=1) as pbig:
            hT = pbig.tile([128, KC2, RPC], bf16)
            M2 = pbig.tile([128, NBLK * EC, 128], bf16)
            gT = pbig.tile([128, 2 * KC2, GPC], bf16)

            # Phase T: hsl kc-slabs -> hT (DMA transposes), build M2 masks
            for kc in range(KC2):
                nc.sync.dma_start_transpose(out=hT[:, kc, :], in_=hsl[kc])
            with tc.tile_pool(name="pm2", bufs=2) as pm2:
                for col in range(NBLK * EC):
                    msk = pm2.tile([128, 128], bf16, tag="m2m")
                    nc.vector.tensor_tensor(
                        out=msk,
                        in0=dlocc_sb[:, col:col + 1].to_broadcast([128, 128]),
                        in1=iota_f, op=mybir.AluOpType.is_equal)
                    nc.vector.tensor_tensor(
                        out=M2[:, col, :], in0=msk,
                        in1=normc_sb[:, col:col + 1].to_broadcast([128, 128]),
                        op=mybir.AluOpType.mult)

            # ---------------- Phase E + segmented AllGather + Phase D -------
            with tc.tile_pool(name="pe_w", bufs=2) as pwp, \
                 tc.tile_pool(name="pe_sb", bufs=2) as pe, \
                 tc.tile_pool(name="pe_xt", bufs=2) as pxt, \
                 tc.tile_pool(name="pe_ps", bufs=2, space="PSUM") as pps3, \
                 tc.tile_pool(name="pd_he", bufs=1) as phe, \
                 tc.tile_pool(name="pd_h2", bufs=1) as ph2pool, \
                 tc.tile_pool(name="pd_sb", bufs=2) as pd, \
                 tc.tile_pool(name="pd_ps", bufs=4, space="PSUM") as ppd:

                def e_segment(s, then=None):
                    """Transform 9 fp chunks; call `then()` before issuing the
                    AllGather so queued gpsimd work (gathers) isn't stuck
                    behind the collective's wait."""
                    for fpi in range(SEGF):
                        fp = SEGF * s + fpi
                        wsl = pwp.tile([128, KC2, 128], bf16, tag="wsl")
                        nc.sync.dma_start(out=wsl, in_=wgcn[fp])
                        ph2 = pps3.tile([128, RPC], f32, tag="ph2")
                        for kc in range(KC2):
                            nc.tensor.matmul(ph2[:, 0:512], wsl[:, kc, :],
                                             hT[:, kc, 0:512],
                                             start=(kc == 0),
                                             stop=(kc == KC2 - 1))
                            nc.tensor.matmul(ph2[:, 512:RPC], wsl[:, kc, :],
                                             hT[:, kc, 512:RPC],
                                             start=(kc == 0),
                                             stop=(kc == KC2 - 1))
                        xwf = pe.tile([128, RPC], bf16, tag="xwf")
                        nc.vector.tensor_copy(xwf, ph2)
                        for nb in range(RPC // 128):
                            xwt = pxt.tile([128, 128], bf16, tag="xwt")
                            nc.sync.dma_start_transpose(
                                out=xwt, in_=xwf[:, 128 * nb:128 * (nb + 1)])
                            nc.sync.dma_start(
                                out=xwsl[s][128 * nb:128 * (nb + 1),
                                            128 * fpi:128 * (fpi + 1)],
                                in_=xwt)
                    if then is not None:
                        then()
                    nc.gpsimd.collective_compute(
                        "AllGather", mybir.AluOpType.bypass,
                        replica_groups=[list(range(NC_))],
                        ins=[xwsl[s]], outs=[xwpad[s]])

                def d_segment(s):
                    h2a = ph2pool.tile([128, SEGF, RPC], bf16, tag="h2a")
                    for b in range(NBLK):
                        hes = []
                        for e in range(EC):
                            col = b * EC + e
                            he = phe.tile([128, SEGW], bf16, tag=f"he{e}")
                            nc.gpsimd.indirect_dma_start(
                                out=he, out_offset=None, in_=xwpad[s],
                                in_offset=bass.IndirectOffsetOnAxis(
                                    ap=esrch_sb[:, col:col + 1], axis=0))
                            hes.append(he)
                        for kci in range(SEGF):
                            p2 = ppd.tile([128, 128], f32, tag="p2")
                            for e in range(EC):
                                nc.tensor.matmul(
                                    p2,
                                    hes[e][:, 128 * kci:128 * (kci + 1)],
                                    M2[:, b * EC + e, :],
                                    start=(e == 0), stop=(e == EC - 1))
                            nc.scalar.activation(
                                out=h2a[:, kci, 128 * b:128 * (b + 1)],
                                in_=p2,
                                func=mybir.ActivationFunctionType.Relu)
                    for kci in range(SEGF):
                        kc = SEGF * s + kci
                        h2r = h2a[:, kci, :NPC].rearrange(
                            "p (g n) -> p g n", n=NPG)
                        gmax = pd.tile([128, GPC], f32, tag="gmax")
                        nc.vector.tensor_reduce(out=gmax, in_=h2r,
                                                axis=mybir.AxisListType.X,
                                                op=mybir.AluOpType.max)
                        gsum = pd.tile([128, GPC], f32, tag="gsum")
                        nc.vector.tensor_reduce(out=gsum, in_=h2r,
                                                axis=mybir.AxisListType.X,
                                                op=mybir.AluOpType.add)
                        nc.vector.tensor_copy(gT[:, kc, :], gmax)
                        nc.scalar.activation(
                            out=gT[:, KC2 + kc, :], in_=gsum,
                            func=mybir.ActivationFunctionType.Copy,
                            scale=1.0 / NPG)

                for s in range(NSEG):
                    e_segment(s, then=(
                        (lambda s=s: d_segment(s - 1)) if s > 0 else None))
                d_segment(NSEG - 1)

            # ---------------- MLP (all transposed) ----------------
            with tc.tile_pool(name="pf_w", bufs=4) as pw1, \
                 tc.tile_pool(name="pf_sb", bufs=2) as pf, \
                 tc.tile_pool(name="pf_p1", bufs=4, space="PSUM") as pp1, \
                 tc.tile_pool(name="pf_p2", bufs=1, space="PSUM") as pp2:
                p1s = [pp1.tile([128, 32], f32, tag="p1", name=f"p1_{i}")
                       for i in range(4)]
                for kc in range(2 * KC2):
                    w1sl = pw1.tile([128, 512], bf16, tag="w1sl")
                    nc.sync.dma_start(out=w1sl, in_=w1t[kc])
                    for mc in range(4):
                        nc.tensor.matmul(
                            p1s[mc], w1sl[:, 128 * mc:128 * (mc + 1)],
                            gT[:, kc, :],
                            start=(kc == 0), stop=(kc == 2 * KC2 - 1))
                o1 = pf.tile([128, 4, 32], bf16, tag="o1")
                for mc in range(4):
                    nc.scalar.activation(
                        out=o1[:, mc, :], in_=p1s[mc],
                        func=mybir.ActivationFunctionType.Relu)
                w2sb = pf.tile([128, 4, 128], bf16, tag="w2sb")
                nc.sync.dma_start(out=w2sb,
                                  in_=w2t.rearrange("c p f -> p c f"))
                p2t = pp2.tile([128, 32], f32, tag="p2t")
                for kc in range(4):
                    nc.tensor.matmul(p2t, w2sb[:, kc, :], o1[:, kc, :],
                                     start=(kc == 0), stop=(kc == 3))
                o2 = pf.tile([128, 32], bf16, tag="o2")
                nc.vector.tensor_copy(o2, p2t)
                w3sb = pf.tile([128, 64], bf16, tag="w3sb")
                nc.sync.dma_start(out=w3sb, in_=w3t)
                p3t = pp2.tile([64, 32], f32, tag="p3t")
                nc.tensor.matmul(p3t, w3sb, o2, start=True, stop=True)
                o3 = pf.tile([64, 32], f32, tag="o3")
                nc.vector.tensor_copy(o3, p3t)
                nc.sync.dma_start(out=outg, in_=o3)

    nc.compile()
    return nc


_NC_CACHE = None


def get_nc():
    global _NC_CACHE
    if _NC_CACHE is None:
        _NC_CACHE = build_nc()
    return _NC_CACHE


def make_in_maps(inputs):
    shared, per_core = host_prep(inputs)
    return [dict(shared, **pc) for pc in per_core]


def kernel(**inputs):
    from concourse.bass_utils import run_bass_kernel_spmd
    nc = get_nc()
    in_maps = make_in_maps(inputs)
    res = run_bass_kernel_spmd(nc, in_maps, core_ids=list(range(NC_)))
    out = np.zeros((G, 64), np.float32)
    for c in range(NC_):
        out[GPC * c:GPC * (c + 1), :] = res.results[c]["outg"].T
    return out


if __name__ == "__main__":
    d = np.load("/root/problem/inputs.npz")
    inputs = {k: d[k] for k in d.files}
    out = kernel(**inputs)
    print("out", out.shape, out.dtype, out[:2, :4])


# revision 19
# speedup vs baseline: 1.0495x; 1.0495x over previous
"""Trainium2 Bass kernel for nn_GAT_GCN (GAT -> GCN -> readout -> MLP), 8-core SPMD.

v2: inverted GCN (transform-then-aggregate) so the big AllGather pipelines
under the 6800x6800 GCN matmul instead of sitting exposed on the critical
path.

Sharding: 1024 rows per core: 992 graph-aligned nodes (32 graphs x 31) plus
32 orphan rows (the 64 readout-dropped nodes 7936..7999 split across cores
6 and 7). Edges are owned by their dst node; 8 dst blocks of 128 per core.

Pipeline per core:
- A: al = x @ [As|Ad] for OWN nodes only; AllGather -> alsp_pad[8192,32].
- B: GAT per dst block (one-hot mask matmul aggregation, softmax 1/z folded
  into the PSUM-drain activation scale); h written to DRAM kc-slab-major.
- T: h -> hT[128,54,1024] (feat-major, SBUF resident) via DMA transposes.
- E: xw = h @ W_gcn on own nodes (fp-chunk loop, wgcn streamed); xw rows
  re-transposed to node-major and AllGathered in 6 column segments, each
  issued as soon as its 9 fp chunks finish -> comm hides under E compute.
- D: GCN aggregation per segment via norm-weighted one-hot masks (gathering
  xw rows per edge), fused relu + graph max/mean readout; interleaved with
  E so segment s aggregates while segment s+1 transforms.
- MLP on [64,32] transposed output; host concatenates.
"""
import sys
import numpy as np
import ml_dtypes

sys.path.insert(0, "/opt/trn_rl_repo")

from contextlib import ExitStack  # noqa: E402

import concourse.bass as bass  # noqa: E402
import concourse.tile as tile  # noqa: E402
from concourse import bacc, mybir  # noqa: E402

N, E, G = 8000, 32000, 256
F, H = 680, 10
HF = F * H                    # 6800
NC_ = 8                       # cores
NPC = 992                     # readout nodes per core (32 graphs x 31)
RPC = 1024                    # rows per core (992 + 32 orphan slots)
NBLK = 8                      # dst blocks per core (128 each)
EB = 768                      # padded edges per block
EC = EB // 128                # 6 edge chunks
FP = 768                      # padded F
KC1 = FP // 128               # 6
HFP = 6912                    # padded HF
KC2 = HFP // 128              # 54
NSEG = 6                      # xw AllGather segments
SEGF = KC2 // NSEG            # 9 fp chunks per segment
SEGW = SEGF * 128             # 1152 cols per segment
GPC = 32                      # graphs per core
NPG = 31                      # nodes per graph

f32 = mybir.dt.float32
bf16 = mybir.dt.bfloat16
i32 = mybir.dt.int32
bfnp = ml_dtypes.bfloat16


# ----------------------------------------------------------------------------
# Host-side prep: sharding, padding, weight tiling
# ----------------------------------------------------------------------------

def node_owner_local(node):
    """node -> (owner core, local row) for the 992x8 + 32/32 orphan layout."""
    node = np.asarray(node)
    owner = np.where(node < 7936, node // NPC,
                     np.where(node < 7968, 6, 7))
    local = np.where(node < 7936, node - NPC * (node // NPC),
                     np.where(node < 7968, NPC + node - 7936,
                              NPC + node - 7968))
    return owner, local


def host_prep(inputs):
    x = np.asarray(inputs["x"], np.float32)
    edge_index = np.asarray(inputs["edge_index"])
    W_gat = np.asarray(inputs["W_gat"], np.float32)
    a_src = np.asarray(inputs["a_src"], np.float32)
    a_dst = np.asarray(inputs["a_dst"], np.float32)
    W_gcn = np.asarray(inputs["W_gcn"], np.float32)
    W1 = np.asarray(inputs["W1"], np.float32)
    W2 = np.asarray(inputs["W2"], np.float32)
    W3 = np.asarray(inputs["W3"], np.float32)
    for bname in ("b_gat", "b_gcn", "b1", "b2", "b3"):
        assert np.all(np.asarray(inputs[bname]) == 0), f"nonzero {bname}"

    src = np.concatenate([edge_index[0], np.arange(N)]).astype(np.int64)
    dst = np.concatenate([edge_index[1], np.arange(N)]).astype(np.int64)
    deg = np.bincount(dst, minlength=N).astype(np.float64)
    dinv = 1.0 / np.sqrt(deg)
    norm = (dinv[src] * dinv[dst]).astype(np.float32)

    owner_n, local_n = node_owner_local(np.arange(N))
    hpos = RPC * owner_n + local_n                       # node -> global row

    xb = np.zeros((N, FP), bfnp)
    xb[:, :F] = x.astype(bfnp)

    As = np.stack([W_gat[:, h * F:(h + 1) * F] @ a_src[h] for h in range(H)], 1)
    Ad = np.stack([W_gat[:, h * F:(h + 1) * F] @ a_dst[h] for h in range(H)], 1)
    ascat = np.zeros((FP, 64), bfnp)
    ascat[:F, :H] = As.astype(bfnp)
    ascat[:F, H:2 * H] = Ad.astype(bfnp)

    wgat = np.zeros((KC1, 128, HF), bfnp)
    wgat.reshape(FP, HF)[:F] = W_gat.astype(bfnp)

    wpad = np.zeros((HFP, HFP), np.float32)
    wpad[:HF, :HF] = W_gcn
    # [fp, i(k row in chunk), kc, j] -> per-partition contiguous DMA slabs
    wgcn = np.ascontiguousarray(
        wpad.reshape(KC2, 128, KC2, 128).transpose(2, 1, 0, 3)).astype(bfnp)

    # gT k-order: 54 gmp chunks (rows [0,HF) + 16 pad) then 54 gap chunks
    w1t = np.zeros((2 * KC2, 128, 512), bfnp)
    w1t.reshape(2 * HFP, 512)[:HF] = W1[:HF].astype(bfnp)
    w1t.reshape(2 * HFP, 512)[HFP:HFP + HF] = W1[HF:].astype(bfnp)
    w2t = np.ascontiguousarray(W2.reshape(4, 128, 128)).astype(bfnp)
    w3t = np.ascontiguousarray(W3).astype(bfnp)              # [128, 64]

    shared = dict(xb=xb, ascat=ascat, wgat=wgat, wgcn=wgcn,
                  w1t=w1t, w2t=w2t, w3t=w3t)

    # per-core own-node x slices (hpos-local row order)
    own_nodes = [np.where(owner_n == c)[0][np.argsort(local_n[owner_n == c])]
                 for c in range(NC_)]

    per_core = []
    for c in range(NC_):
        nodes_c = own_nodes[c]
        xown = np.zeros((RPC, FP), bfnp)
        xown[local_n[nodes_c]] = xb[nodes_c]

        esrcx = np.zeros((NBLK, EC, 128), np.int32)
        esrch = np.zeros((NBLK, EC, 128), np.int32)
        eldst = np.zeros((NBLK, EC, 128), np.int32)
        dlocc = np.full((NBLK, EC, 128), -1.0, np.float32)
        normc = np.zeros((NBLK, EC, 128), np.float32)
        em = (owner_n[dst] == c)
        es, ed, en = src[em], dst[em], norm[em]
        loc = local_n[ed]
        for b in range(NBLK):
            bm = (loc >= 128 * b) & (loc < 128 * (b + 1))
            cnt = int(bm.sum())
            assert cnt <= EB, (c, b, cnt)
            fs = np.zeros(EB, np.int64)
            fd = np.zeros(EB, np.int64)
            fl = np.full(EB, -1.0, np.float32)
            fn = np.zeros(EB, np.float32)
            fs[:cnt] = es[bm]
            fd[:cnt] = ed[bm]
            fl[:cnt] = (loc[bm] - 128 * b).astype(np.float32)
            fn[:cnt] = en[bm]
            esrcx[b] = fs.reshape(EC, 128)
            esrch[b] = hpos[fs].reshape(EC, 128)
            eldst[b] = hpos[fd].reshape(EC, 128)
            dlocc[b] = fl.reshape(EC, 128)
            normc[b] = fn.reshape(EC, 128)
        pc = dict(
            xown=xown,
            esrcx=np.ascontiguousarray(esrcx.reshape(NBLK * EC, 128).T),
            esrch=np.ascontiguousarray(esrch.reshape(NBLK * EC, 128).T),
            eldst=np.ascontiguousarray(eldst.reshape(NBLK * EC, 128).T),
            dlocc=np.ascontiguousarray(dlocc.reshape(NBLK * EC, 128).T),
            normc=np.ascontiguousarray(
                normc.reshape(NBLK * EC, 128).T.astype(bfnp)),
        )
        per_core.append(pc)
    return shared, per_core


# ----------------------------------------------------------------------------
# Device program (one SPMD Bass program; all per-core variation is via data)
# ----------------------------------------------------------------------------

def build_nc():
    nc = bacc.Bacc("TRN2", target_bir_lowering=False, debug=False,
                   num_devices=NC_)
    xb = nc.dram_tensor("xb", [N, FP], bf16, kind="ExternalInput").ap()
    xown = nc.dram_tensor("xown", [RPC, FP], bf16, kind="ExternalInput").ap()
    ascat = nc.dram_tensor("ascat", [FP, 64], bf16, kind="ExternalInput").ap()
    wgat = nc.dram_tensor("wgat", [KC1, 128, HF], bf16, kind="ExternalInput").ap()
    wgcn = nc.dram_tensor("wgcn", [KC2, 128, KC2, 128], bf16,
                          kind="ExternalInput").ap()
    w1t = nc.dram_tensor("w1t", [2 * KC2, 128, 512], bf16,
                         kind="ExternalInput").ap()
    w2t = nc.dram_tensor("w2t", [4, 128, 128], bf16, kind="ExternalInput").ap()
    w3t = nc.dram_tensor("w3t", [128, 64], bf16, kind="ExternalInput").ap()
    esrcx = nc.dram_tensor("esrcx", [128, NBLK * EC], i32,
                           kind="ExternalInput").ap()
    esrch = nc.dram_tensor("esrch", [128, NBLK * EC], i32,
                           kind="ExternalInput").ap()
    eldst = nc.dram_tensor("eldst", [128, NBLK * EC], i32,
                           kind="ExternalInput").ap()
    dlocc = nc.dram_tensor("dlocc", [128, NBLK * EC], f32,
                           kind="ExternalInput").ap()
    normc = nc.dram_tensor("normc", [128, NBLK * EC], bf16,
                           kind="ExternalInput").ap()
    outg = nc.dram_tensor("outg", [64, 32], f32, kind="ExternalOutput").ap()

    with tile.TileContext(nc) as tc, ExitStack() as ctx:
        dram = ctx.enter_context(tc.tile_pool(name="dram", bufs=1, space="DRAM"))
        alsl = dram.tile([RPC, 32], f32, name="alsl")
        alsp = dram.tile([NC_ * RPC, 32], f32, name="alsp", addr_space="Shared")
        hsl = dram.tile([KC2, RPC, 128], bf16, name="hsl")
        xwsl = [dram.tile([RPC, SEGW], bf16, name=f"xwsl{s}")
                for s in range(NSEG)]
        xwpad = [dram.tile([NC_ * RPC, SEGW], bf16, name=f"xwpad{s}",
                           addr_space="Shared") for s in range(NSEG)]
        singles = ctx.enter_context(tc.tile_pool(name="singles", bufs=1))

        iota_i = singles.tile([128, 128], i32)
        nc.gpsimd.iota(iota_i, pattern=[[1, 128]], base=0, channel_multiplier=0)
        iota_f = singles.tile([128, 128], f32)
        nc.vector.tensor_copy(iota_f, iota_i)

        ascat_sb = singles.tile([128, KC1, 64], bf16)
        nc.sync.dma_start(out=ascat_sb,
                          in_=ascat.rearrange("(c p) d -> p c d", p=128))
        esrcx_sb = singles.tile([128, NBLK * EC], i32)
        nc.sync.dma_start(out=esrcx_sb, in_=esrcx)
        esrch_sb = singles.tile([128, NBLK * EC], i32)
        nc.sync.dma_start(out=esrch_sb, in_=esrch)
        eldst_sb = singles.tile([128, NBLK * EC], i32)
        nc.sync.dma_start(out=eldst_sb, in_=eldst)
        dlocc_sb = singles.tile([128, NBLK * EC], f32)
        nc.sync.dma_start(out=dlocc_sb, in_=dlocc)
        normc_sb = singles.tile([128, NBLK * EC], bf16)
        nc.sync.dma_start(out=normc_sb, in_=normc)

        from concourse.masks import make_identity
        ident = singles.tile([128, 128], bf16, name="ident")
        make_identity(nc, ident)

        # ---------------- Phase A: al = x_own @ [As|Ad] -> AllGather ----------
        with tc.tile_pool(name="pa_sb", bufs=3) as pool, \
             tc.tile_pool(name="pa_ps", bufs=2, space="PSUM") as pps, \
             tc.tile_pool(name="pa_pt", bufs=4, space="PSUM") as ppt:
            for i in range(RPC // 128):
                r0 = 128 * i
                xr = pool.tile([128, FP], bf16, tag="xr")
                nc.sync.dma_start(out=xr, in_=xown[r0:r0 + 128, :])
                xt = pool.tile([128, KC1, 128], bf16, tag="xt")
                for k in range(KC1):
                    pt = ppt.tile([128, 128], bf16, tag="pt")
                    nc.tensor.transpose(
                        out=pt, in_=xr[:, 128 * k:128 * (k + 1)],
                        identity=ident)
                    nc.vector.tensor_copy(xt[:, k, :], pt)
                pal = pps.tile([128, 2 * H], f32, tag="pal")
                for k in range(KC1):
                    nc.tensor.matmul(pal, xt[:, k, :],
                                     ascat_sb[:, k, :2 * H],
                                     start=(k == 0), stop=(k == KC1 - 1))
                al_sb = pool.tile([128, 32], f32, tag="al")
                nc.vector.tensor_copy(al_sb[:, :2 * H], pal)
                nc.vector.memset(al_sb[:, 2 * H:], 0.0)
                nc.sync.dma_start(out=alsl[r0:r0 + 128, :], in_=al_sb)
        nc.gpsimd.collective_compute(
            "AllGather", mybir.AluOpType.bypass,
            replica_groups=[list(range(NC_))],
            ins=[alsl], outs=[alsp])

        # ---------------- Phase B: GAT blocks -> hsl (kc-slab-major) ---------
        with tc.tile_pool(name="pb_w", bufs=1) as pw, \
             tc.tile_pool(name="pb_sb", bufs=2) as pool, \
             tc.tile_pool(name="pb_sm", bufs=3) as psm, \
             tc.tile_pool(name="pb_m", bufs=EC + 2) as pm, \
             tc.tile_pool(name="pb_ps", bufs=2, space="PSUM") as pps, \
             tc.tile_pool(name="pb_ph", bufs=1, space="PSUM") as pph, \
             tc.tile_pool(name="pb_pz", bufs=2, space="PSUM") as ppz:
            wgat_sb = pw.tile([128, KC1, HF], bf16)
            for k in range(KC1):
                nc.sync.dma_start(out=wgat_sb[:, k, :], in_=wgat[k])

            for b in range(NBLK):
                xe = pool.tile([128, EC, FP], bf16, tag="xe")
                als = pool.tile([128, EC, 32], f32, tag="als")
                ald = pool.tile([128, EC, 32], f32, tag="ald")
                for e in range(EC):
                    col = b * EC + e
                    nc.gpsimd.indirect_dma_start(
                        out=xe[:, e, :], out_offset=None, in_=xb,
                        in_offset=bass.IndirectOffsetOnAxis(
                            ap=esrcx_sb[:, col:col + 1], axis=0))
                    nc.gpsimd.indirect_dma_start(
                        out=als[:, e, :], out_offset=None, in_=alsp,
                        in_offset=bass.IndirectOffsetOnAxis(
                            ap=esrch_sb[:, col:col + 1], axis=0))
                    nc.gpsimd.indirect_dma_start(
                        out=ald[:, e, :], out_offset=None, in_=alsp,
                        in_offset=bass.IndirectOffsetOnAxis(
                            ap=eldst_sb[:, col:col + 1], axis=0))

                masks = []
                exb = psm.tile([128, EC, H], bf16, tag="exb")
                for e in range(EC):
                    col = b * EC + e
                    msk = pm.tile([128, 128], bf16, tag="msk")
                    nc.vector.tensor_tensor(
                        out=msk,
                        in0=dlocc_sb[:, col:col + 1].to_broadcast([128, 128]),
                        in1=iota_f, op=mybir.AluOpType.is_equal)
                    masks.append(msk)
                    # logits -> exp (leaky_relu slope 0.2)
                    lg = psm.tile([128, H], f32, tag="lg")
                    nc.vector.tensor_tensor(out=lg, in0=als[:, e, :H],
                                            in1=ald[:, e, H:2 * H],
                                            op=mybir.AluOpType.add)
                    lg2 = psm.tile([128, H], f32, tag="lg2")
                    nc.vector.tensor_scalar_mul(lg2, lg, 0.2)
                    nc.vector.tensor_tensor(out=lg, in0=lg, in1=lg2,
                                            op=mybir.AluOpType.max)
                    nc.scalar.activation(out=exb[:, e, :], in_=lg,
                                         func=mybir.ActivationFunctionType.Exp)

                # z[d,h] = sum_e mask[e,d] * ex[e,h]
                pz = ppz.tile([128, H], f32, tag="pz")
                for e in range(EC):
                    nc.tensor.matmul(pz, masks[e], exb[:, e, :],
                                     start=(e == 0), stop=(e == EC - 1))
                zf = psm.tile([128, H], f32, tag="zf")
                nc.scalar.activation(out=zf, in_=pz,
                                     func=mybir.ActivationFunctionType.Copy,
                                     bias=1e-30)
                zinv = psm.tile([128, H], f32, tag="zinv")
                nc.vector.reciprocal(zinv, zf)

                # M_e[:, h, :] = mask_e * ex[e, h]  (vector/scalar split)
                aggT = pool.tile([128, KC1, H, 128], bf16, tag="aggT")
                Ms = []
                for e in range(EC):
                    Me = pm.tile([128, H, 128], bf16, tag="Me")
                    for h in range(H):
                        nc.vector.tensor_tensor(
                            out=Me[:, h, :], in0=masks[e],
                            in1=exb[:, e, h:h + 1].to_broadcast([128, 128]),
                            op=mybir.AluOpType.mult)
                    Ms.append(Me)
                # aggT[f, (h d)] += xe.T @ M
                for k in range(KC1):
                    for half in range(2):
                        pa = pps.tile([128, 5 * 128], f32, tag="pa")
                        h0 = 5 * half
                        for e in range(EC):
                            lhs = xe[:, e, 128 * k:128 * (k + 1)]
                            nc.tensor.matmul(pa[:, 0:512], lhs,
                                             Ms[e][:, h0:h0 + 4, :],
                                             start=(e == 0), stop=(e == EC - 1))
                            nc.tensor.matmul(pa[:, 512:640], lhs,
                                             Ms[e][:, h0 + 4:h0 + 5, :],
                                             start=(e == 0), stop=(e == EC - 1))
                        nc.vector.tensor_copy(aggT[:, k, h0:h0 + 5, :], pa)

                # transform per head + fused 1/z scale + relu
                h1 = pool.tile([128, HFP], bf16, tag="h1")
                for h in range(H):
                    ph = pph.tile([128, F], f32, tag="ph")
                    for k in range(KC1):
                        lhs = aggT[:, k, h, :]
                        nc.tensor.matmul(ph[:, 0:512], lhs,
                                         wgat_sb[:, k, F * h:F * h + 512],
                                         start=(k == 0), stop=(k == KC1 - 1))
                        nc.tensor.matmul(ph[:, 512:F], lhs,
                                         wgat_sb[:, k, F * h + 512:F * (h + 1)],
                                         start=(k == 0), stop=(k == KC1 - 1))
                    nc.scalar.activation(out=h1[:, F * h:F * (h + 1)], in_=ph,
                                         func=mybir.ActivationFunctionType.Relu,
                                         scale=zinv[:, h:h + 1])
                nc.vector.memset(h1[:, HF:HFP], 0.0)
                for kc in range(KC2):
                    nc.sync.dma_start(
                        out=hsl[kc, 128 * b:128 * (b + 1), :],
                        in_=h1[:, 128 * kc:128 * (kc + 1)])

        # ---------------- big resident tiles for T/E/D ----------------
        with tc.tile_pool(name="pe_big", bufs=1) as pbig:
            hTs = [pbig.tile([128, RPC], bf16, name=f"hT{kc}")
                   for kc in range(KC2)]
            gT = pbig.tile([128, 2 * KC2, GPC], bf16)

            # Phase T: hsl kc-slabs -> hT (DMA transposes); E chases these
            for kc in range(KC2):
                nc.sync.dma_start_transpose(out=hTs[kc], in_=hsl[kc])

            # --- Phase E + segmented AllGather + Phase D, interleaved ------
            # PE queue order per segment s:
            #   [E fp0][E fp1][D(s-1) b0][E fp2][D b1] ... [E fp8][D b7]
            #   [readout(s-1) + MLP acc][AllGather(s)]
            # so D matmuls (gather-dependent) never head-of-line-block E.
            with tc.tile_pool(name="pe_w", bufs=2) as pwp, \
                 tc.tile_pool(name="pe_sb", bufs=2) as pe, \
                 tc.tile_pool(name="pe_xt", bufs=2) as pxt, \
                 tc.tile_pool(name="pe_ps", bufs=2, space="PSUM") as pps3, \
                 tc.tile_pool(name="pd_he", bufs=2) as phe, \
                 tc.tile_pool(name="pd_m", bufs=2) as pmsk, \
                 tc.tile_pool(name="pd_h2", bufs=1) as ph2pool, \
                 tc.tile_pool(name="pd_sb", bufs=2) as pd, \
                 tc.tile_pool(name="pd_ps", bufs=2, space="PSUM") as ppd, \
                 tc.tile_pool(name="pf_w", bufs=4) as pw1, \
                 tc.tile_pool(name="pf_p1", bufs=1, space="PSUM") as pp1:
                p1acc = pp1.tile([32, 512], f32, name="p1acc")
                h2as = {}
                dstate = {}

                def e_fp(s, fpi):
                    fp = SEGF * s + fpi
                    wsl = pwp.tile([128, KC2, 128], bf16, tag="wsl")
                    nc.sync.dma_start(out=wsl, in_=wgcn[fp])
                    ph2 = pps3.tile([128, RPC], f32, tag="ph2")
                    for kc in range(KC2):
                        nc.tensor.matmul(ph2[:, 0:512], wsl[:, kc, :],
                                         hTs[kc][:, 0:512],
                                         start=(kc == 0), stop=(kc == KC2 - 1))
                        nc.tensor.matmul(ph2[:, 512:RPC], wsl[:, kc, :],
                                         hTs[kc][:, 512:RPC],
                                         start=(kc == 0), stop=(kc == KC2 - 1))
                    xwf = pe.tile([128, RPC], bf16, tag="xwf")
                    nc.vector.tensor_copy(xwf, ph2)
                    for nb in range(RPC // 128):
                        xwt = pxt.tile([128, 128], bf16, tag="xwt")
                        nc.sync.dma_start_transpose(
                            out=xwt, in_=xwf[:, 128 * nb:128 * (nb + 1)])
                        nc.sync.dma_start(
                            out=xwsl[s][128 * nb:128 * (nb + 1),
                                        128 * fpi:128 * (fpi + 1)],
                            in_=xwt)

                def d_gathers(s, b):
                    """Prefetch block b's edge rows + masks (no PE work)."""
                    hes, m2s = [], []
                    for e in range(EC):
                        col = b * EC + e
                        he = phe.tile([128, SEGW], bf16, tag=f"he{e}")
                        nc.gpsimd.indirect_dma_start(
                            out=he, out_offset=None, in_=xwpad[s],
                            in_offset=bass.IndirectOffsetOnAxis(
                                ap=esrch_sb[:, col:col + 1], axis=0))
                        hes.append(he)
                        msk = pmsk.tile([128, 128], bf16, tag=f"mk{e}")
                        nc.vector.tensor_tensor(
                            out=msk,
                            in0=dlocc_sb[:, col:col + 1].to_broadcast(
                                [128, 128]),
                            in1=iota_f, op=mybir.AluOpType.is_equal)
                        nc.vector.tensor_tensor(
                            out=msk, in0=msk,
                            in1=normc_sb[:, col:col + 1].to_broadcast(
                                [128, 128]),
                            op=mybir.AluOpType.mult)
                        m2s.append(msk)
                    dstate[(s, b)] = (hes, m2s)

                def d_block(s, b):
                    """Aggregation matmuls + relu drains for block b."""
                    hes, m2s = dstate.pop((s, b))
                    h2a = h2as[s]
                    for kci in range(SEGF):
                        p2 = ppd.tile([128, 128], f32, tag="p2")
                        for e in range(EC):
                            nc.tensor.matmul(
                                p2, hes[e][:, 128 * kci:128 * (kci + 1)],
                                m2s[e],
                                start=(e == 0), stop=(e == EC - 1))
                        nc.scalar.activation(
                            out=h2a[:, kci, 128 * b:128 * (b + 1)], in_=p2,
                            func=mybir.ActivationFunctionType.Relu)

                def d_readout(s):
                    """Graph max/mean per kc + fold in the W1 MLP matmuls."""
                    h2a = h2as.pop(s)
                    for kci in range(SEGF):
                        kc = SEGF * s + kci
                        h2r = h2a[:, kci, :NPC].rearrange(
                            "p (g n) -> p g n", n=NPG)
                        gmax = pd.tile([128, GPC], f32, tag="gmax")
                        nc.vector.tensor_reduce(out=gmax, in_=h2r,
                                                axis=mybir.AxisListType.X,
                                                op=mybir.AluOpType.max)
                        gsum = pd.tile([128, GPC], f32, tag="gsum")
                        nc.vector.tensor_reduce(out=gsum, in_=h2r,
                                                axis=mybir.AxisListType.X,
                                                op=mybir.AluOpType.add)
                        nc.vector.tensor_copy(gT[:, kc, :], gmax)
                        nc.scalar.activation(
                            out=gT[:, KC2 + kc, :], in_=gsum,
                            func=mybir.ActivationFunctionType.Copy,
                            scale=1.0 / NPG)
                        for part, gk in ((0, kc), (1, KC2 + kc)):
                            w1sl = pw1.tile([128, 512], bf16, tag="w1sl")
                            nc.sync.dma_start(out=w1sl, in_=w1t[gk])
                            nc.tensor.matmul(
                                p1acc, gT[:, gk, :], w1sl,
                                start=(kc == 0 and part == 0),
                                stop=(kc == KC2 - 1 and part == 1))

                for s in range(NSEG):
                    if s > 0:
                        h2as[s - 1] = ph2pool.tile([128, SEGF, RPC], bf16,
                                                   tag="h2a", name="h2a")
                    for fpi in range(SEGF):
                        e_fp(s, fpi)
                        if s > 0:
                            if fpi < NBLK:
                                d_gathers(s - 1, fpi)
                            if fpi >= 1:
                                d_block(s - 1, fpi - 1)
                    if s > 0:
                        d_readout(s - 1)
                    nc.gpsimd.collective_compute(
                        "AllGather", mybir.AluOpType.bypass,
                        replica_groups=[list(range(NC_))],
                        ins=[xwsl[s]], outs=[xwpad[s]])
                # last segment's aggregation + readout (tail)
                h2as[NSEG - 1] = ph2pool.tile([128, SEGF, RPC], bf16,
                                              tag="h2a", name="h2a")
                for b in range(NBLK):
                    d_gathers(NSEG - 1, b)
                    if b >= 1:
                        d_block(NSEG - 1, b - 1)
                d_block(NSEG - 1, NBLK - 1)
                d_readout(NSEG - 1)

                # ---------------- MLP tail ----------------
                with tc.tile_pool(name="pf_sb", bufs=2) as pf:
                    o1g = pf.tile([32, 512], bf16, tag="o1g")
                    nc.scalar.activation(
                        out=o1g, in_=p1acc,
                        func=mybir.ActivationFunctionType.Relu)
                    o1 = pf.tile([128, 4, 32], bf16, tag="o1")
                    for mc in range(4):
                        pt1 = ppd.tile([128, 32], bf16, tag="p2")
                        nc.tensor.transpose(
                            out=pt1, in_=o1g[:, 128 * mc:128 * (mc + 1)],
                            identity=ident[:32, :32])
                        nc.vector.tensor_copy(o1[:, mc, :], pt1)
                    w2sb = pf.tile([128, 4, 128], bf16, tag="w2sb")
                    nc.sync.dma_start(out=w2sb,
                                      in_=w2t.rearrange("c p f -> p c f"))
                    p2t = ppd.tile([128, 32], f32, tag="p2")
                    for kc in range(4):
                        nc.tensor.matmul(p2t, w2sb[:, kc, :], o1[:, kc, :],
                                         start=(kc == 0), stop=(kc == 3))
                    o2 = pf.tile([128, 32], bf16, tag="o2")
                    nc.vector.tensor_copy(o2, p2t)
                    w3sb = pf.tile([128, 64], bf16, tag="w3sb")
                    nc.sync.dma_start(out=w3sb, in_=w3t)
                    p3t = ppd.tile([64, 32], f32, tag="p2")
                    nc.tensor.matmul(p3t, w3sb, o2, start=True, stop=True)
                    o3 = pf.tile([64, 32], f32, tag="o3")
                    nc.vector.tensor_copy(o3, p3t)
                    nc.sync.dma_start(out=outg, in_=o3)

    nc.compile()
    return nc


_NC_CACHE = None


def get_nc():
    global _NC_CACHE
    if _NC_CACHE is None:
        _NC_CACHE = build_nc()
    return _NC_CACHE


def make_in_maps(inputs):
    shared, per_core = host_prep(inputs)
    return [dict(shared, **pc) for pc in per_core]


def kernel(**inputs):
    from concourse.bass_utils import run_bass_kernel_spmd
    nc = get_nc()
    in_maps = make_in_maps(inputs)
    res = run_bass_kernel_spmd(nc, in_maps, core_ids=list(range(NC_)))
    out = np.zeros((G, 64), np.float32)
    for c in range(NC_):
        out[GPC * c:GPC * (c + 1), :] = res.results[c]["outg"].T
    return out


if __name__ == "__main__":
    d = np.load("/root/problem/inputs.npz")
    inputs = {k: d[k] for k in d.files}
    out = kernel(**inputs)
    print("out", out.shape, out.dtype, out[:2, :4])


# revision 22
# speedup vs baseline: 1.1457x; 1.0917x over previous
"""Trainium2 Bass kernel for nn_GAT_GCN (GAT -> GCN -> readout -> MLP), 8-core SPMD.

v2: inverted GCN (transform-then-aggregate) so the big AllGather pipelines
under the 6800x6800 GCN matmul instead of sitting exposed on the critical
path.

Sharding: 1024 rows per core: 992 graph-aligned nodes (32 graphs x 31) plus
32 orphan rows (the 64 readout-dropped nodes 7936..7999 split across cores
6 and 7). Edges are owned by their dst node; 8 dst blocks of 128 per core.

Pipeline per core:
- A: al = x @ [As|Ad] for OWN nodes only; AllGather -> alsp_pad[8192,32].
- B: GAT per dst block (one-hot mask matmul aggregation, softmax 1/z folded
  into the PSUM-drain activation scale); h written to DRAM kc-slab-major.
- T: h -> hT[128,54,1024] (feat-major, SBUF resident) via DMA transposes.
- E: xw = h @ W_gcn on own nodes (fp-chunk loop, wgcn streamed); xw rows
  re-transposed to node-major and AllGathered in 6 column segments, each
  issued as soon as its 9 fp chunks finish -> comm hides under E compute.
- D: GCN aggregation per segment via norm-weighted one-hot masks (gathering
  xw rows per edge), fused relu + graph max/mean readout; interleaved with
  E so segment s aggregates while segment s+1 transforms.
- MLP on [64,32] transposed output; host concatenates.
"""
import sys
import numpy as np
import ml_dtypes

sys.path.insert(0, "/opt/trn_rl_repo")

from contextlib import ExitStack  # noqa: E402

import concourse.bass as bass  # noqa: E402
import concourse.tile as tile  # noqa: E402
from concourse import bacc, mybir  # noqa: E402

N, E, G = 8000, 32000, 256
F, H = 680, 10
HF = F * H                    # 6800
NC_ = 8                       # cores
NPC = 992                     # readout nodes per core (32 graphs x 31)
RPC = 1024                    # rows per core (992 + 32 orphan slots)
NBLK = 8                      # dst blocks per core (128 each)
EB = 768                      # padded edges per block
EC = EB // 128                # 6 edge chunks
FP = 768                      # padded F
KC1 = FP // 128               # 6
HFP = 6912                    # padded HF
KC2 = HFP // 128              # 54
NSEG = 6                      # xw AllGather segments
SEGF = KC2 // NSEG            # 9 fp chunks per segment
SEGW = SEGF * 128             # 1152 cols per segment
GPC = 32                      # graphs per core
NPG = 31                      # nodes per graph

f32 = mybir.dt.float32
bf16 = mybir.dt.bfloat16
i32 = mybir.dt.int32
bfnp = ml_dtypes.bfloat16


# ----------------------------------------------------------------------------
# Host-side prep: sharding, padding, weight tiling
# ----------------------------------------------------------------------------

def node_owner_local(node):
    """node -> (owner core, local row) for the 992x8 + 32/32 orphan layout."""
    node = np.asarray(node)
    owner = np.where(node < 7936, node // NPC,
                     np.where(node < 7968, 6, 7))
    local = np.where(node < 7936, node - NPC * (node // NPC),
                     np.where(node < 7968, NPC + node - 7936,
                              NPC + node - 7968))
    return owner, local


def host_prep(inputs):
    x = np.asarray(inputs["x"], np.float32)
    edge_index = np.asarray(inputs["edge_index"])
    W_gat = np.asarray(inputs["W_gat"], np.float32)
    a_src = np.asarray(inputs["a_src"], np.float32)
    a_dst = np.asarray(inputs["a_dst"], np.float32)
    W_gcn = np.asarray(inputs["W_gcn"], np.float32)
    W1 = np.asarray(inputs["W1"], np.float32)
    W2 = np.asarray(inputs["W2"], np.float32)
    W3 = np.asarray(inputs["W3"], np.float32)
    for bname in ("b_gat", "b_gcn", "b1", "b2", "b3"):
        assert np.all(np.asarray(inputs[bname]) == 0), f"nonzero {bname}"

    src = np.concatenate([edge_index[0], np.arange(N)]).astype(np.int64)
    dst = np.concatenate([edge_index[1], np.arange(N)]).astype(np.int64)
    deg = np.bincount(dst, minlength=N).astype(np.float64)
    dinv = 1.0 / np.sqrt(deg)
    norm = (dinv[src] * dinv[dst]).astype(np.float32)

    owner_n, local_n = node_owner_local(np.arange(N))
    hpos = RPC * owner_n + local_n                       # node -> global row

    xb = np.zeros((N, FP), bfnp)
    xb[:, :F] = x.astype(bfnp)

    As = np.stack([W_gat[:, h * F:(h + 1) * F] @ a_src[h] for h in range(H)], 1)
    Ad = np.stack([W_gat[:, h * F:(h + 1) * F] @ a_dst[h] for h in range(H)], 1)
    ascat = np.zeros((FP, 64), bfnp)
    ascat[:F, :H] = As.astype(bfnp)
    ascat[:F, H:2 * H] = Ad.astype(bfnp)

    wgat = np.zeros((KC1, 128, HF), bfnp)
    wgat.reshape(FP, HF)[:F] = W_gat.astype(bfnp)

    wpad = np.zeros((HFP, HFP), np.float32)
    wpad[:HF, :HF] = W_gcn
    # [fp, i(k row in chunk), kc, j] -> per-partition contiguous DMA slabs
    wgcn = np.ascontiguousarray(
        wpad.reshape(KC2, 128, KC2, 128).transpose(2, 1, 0, 3)).astype(bfnp)

    # gT k-order: 54 gmp chunks (rows [0,HF) + 16 pad) then 54 gap chunks
    w1t = np.zeros((2 * KC2, 128, 512), bfnp)
    w1t.reshape(2 * HFP, 512)[:HF] = W1[:HF].astype(bfnp)
    w1t.reshape(2 * HFP, 512)[HFP:HFP + HF] = W1[HF:].astype(bfnp)
    w2t = np.ascontiguousarray(W2.reshape(4, 128, 128)).astype(bfnp)
    w3t = np.ascontiguousarray(W3).astype(bfnp)              # [128, 64]

    shared = dict(xb=xb, ascat=ascat, wgat=wgat, wgcn=wgcn,
                  w1t=w1t, w2t=w2t, w3t=w3t)

    # per-core own-node x slices (hpos-local row order)
    own_nodes = [np.where(owner_n == c)[0][np.argsort(local_n[owner_n == c])]
                 for c in range(NC_)]

    per_core = []
    for c in range(NC_):
        nodes_c = own_nodes[c]
        xown = np.zeros((RPC, FP), bfnp)
        xown[local_n[nodes_c]] = xb[nodes_c]

        esrcx = np.zeros((NBLK, EC, 128), np.int32)
        esrch = np.zeros((NBLK, EC, 128), np.int32)
        eldst = np.zeros((NBLK, EC, 128), np.int32)
        dlocc = np.full((NBLK, EC, 128), -1.0, np.float32)
        normc = np.zeros((NBLK, EC, 128), np.float32)
        em = (owner_n[dst] == c)
        es, ed, en = src[em], dst[em], norm[em]
        loc = local_n[ed]
        for b in range(NBLK):
            bm = (loc >= 128 * b) & (loc < 128 * (b + 1))
            cnt = int(bm.sum())
            assert cnt <= EB, (c, b, cnt)
            fs = np.zeros(EB, np.int64)
            fd = np.zeros(EB, np.int64)
            fl = np.full(EB, -1.0, np.float32)
            fn = np.zeros(EB, np.float32)
            fs[:cnt] = es[bm]
            fd[:cnt] = ed[bm]
            fl[:cnt] = (loc[bm] - 128 * b).astype(np.float32)
            fn[:cnt] = en[bm]
            esrcx[b] = fs.reshape(EC, 128)
            esrch[b] = hpos[fs].reshape(EC, 128)
            eldst[b] = hpos[fd].reshape(EC, 128)
            dlocc[b] = fl.reshape(EC, 128)
            normc[b] = fn.reshape(EC, 128)
        pc = dict(
            xown=xown,
            esrcx=np.ascontiguousarray(esrcx.reshape(NBLK * EC, 128).T),
            esrch=np.ascontiguousarray(esrch.reshape(NBLK * EC, 128).T),
            eldst=np.ascontiguousarray(eldst.reshape(NBLK * EC, 128).T),
            dlocc=np.ascontiguousarray(dlocc.reshape(NBLK * EC, 128).T),
            normc=np.ascontiguousarray(
                normc.reshape(NBLK * EC, 128).T.astype(bfnp)),
        )
        per_core.append(pc)
    return shared, per_core


# ----------------------------------------------------------------------------
# Device program (one SPMD Bass program; all per-core variation is via data)
# ----------------------------------------------------------------------------

def build_nc():
    nc = bacc.Bacc("TRN2", target_bir_lowering=False, debug=False,
                   num_devices=NC_)
    xb = nc.dram_tensor("xb", [N, FP], bf16, kind="ExternalInput").ap()
    xown = nc.dram_tensor("xown", [RPC, FP], bf16, kind="ExternalInput").ap()
    ascat = nc.dram_tensor("ascat", [FP, 64], bf16, kind="ExternalInput").ap()
    wgat = nc.dram_tensor("wgat", [KC1, 128, HF], bf16, kind="ExternalInput").ap()
    wgcn = nc.dram_tensor("wgcn", [KC2, 128, KC2, 128], bf16,
                          kind="ExternalInput").ap()
    w1t = nc.dram_tensor("w1t", [2 * KC2, 128, 512], bf16,
                         kind="ExternalInput").ap()
    w2t = nc.dram_tensor("w2t", [4, 128, 128], bf16, kind="ExternalInput").ap()
    w3t = nc.dram_tensor("w3t", [128, 64], bf16, kind="ExternalInput").ap()
    esrcx = nc.dram_tensor("esrcx", [128, NBLK * EC], i32,
                           kind="ExternalInput").ap()
    esrch = nc.dram_tensor("esrch", [128, NBLK * EC], i32,
                           kind="ExternalInput").ap()
    eldst = nc.dram_tensor("eldst", [128, NBLK * EC], i32,
                           kind="ExternalInput").ap()
    dlocc = nc.dram_tensor("dlocc", [128, NBLK * EC], f32,
                           kind="ExternalInput").ap()
    normc = nc.dram_tensor("normc", [128, NBLK * EC], bf16,
                           kind="ExternalInput").ap()
    outg = nc.dram_tensor("outg", [64, 32], f32, kind="ExternalOutput").ap()

    with tile.TileContext(nc) as tc, ExitStack() as ctx:
        dram = ctx.enter_context(tc.tile_pool(name="dram", bufs=1, space="DRAM"))
        alsl = dram.tile([RPC, 32], f32, name="alsl")
        alsp = dram.tile([NC_ * RPC, 32], f32, name="alsp", addr_space="Shared")
        hsl = dram.tile([KC2, RPC, 128], bf16, name="hsl")
        xwsl = [dram.tile([RPC, SEGW], bf16, name=f"xwsl{s}")
                for s in range(NSEG)]
        xwpad = [dram.tile([NC_ * RPC, SEGW], bf16, name=f"xwpad{s}",
                           addr_space="Shared") for s in range(NSEG)]
        singles = ctx.enter_context(tc.tile_pool(name="singles", bufs=1))

        iota_i = singles.tile([128, 128], i32)
        nc.gpsimd.iota(iota_i, pattern=[[1, 128]], base=0, channel_multiplier=0)
        iota_f = singles.tile([128, 128], f32)
        nc.vector.tensor_copy(iota_f, iota_i)

        ascat_sb = singles.tile([128, KC1, 64], bf16)
        nc.sync.dma_start(out=ascat_sb,
                          in_=ascat.rearrange("(c p) d -> p c d", p=128))
        esrcx_sb = singles.tile([128, NBLK * EC], i32)
        nc.sync.dma_start(out=esrcx_sb, in_=esrcx)
        esrch_sb = singles.tile([128, NBLK * EC], i32)
        nc.sync.dma_start(out=esrch_sb, in_=esrch)
        eldst_sb = singles.tile([128, NBLK * EC], i32)
        nc.sync.dma_start(out=eldst_sb, in_=eldst)
        dlocc_sb = singles.tile([128, NBLK * EC], f32)
        nc.sync.dma_start(out=dlocc_sb, in_=dlocc)
        normc_sb = singles.tile([128, NBLK * EC], bf16)
        nc.sync.dma_start(out=normc_sb, in_=normc)

        from concourse.masks import make_identity
        ident = singles.tile([128, 128], bf16, name="ident")
        make_identity(nc, ident)

        # ---------------- Phase A: al = x_own @ [As|Ad] -> AllGather ----------
        with tc.tile_pool(name="pa_sb", bufs=3) as pool, \
             tc.tile_pool(name="pa_ps", bufs=2, space="PSUM") as pps, \
             tc.tile_pool(name="pa_pt", bufs=4, space="PSUM") as ppt:
            for i in range(RPC // 128):
                r0 = 128 * i
                xr = pool.tile([128, FP], bf16, tag="xr")
                nc.sync.dma_start(out=xr, in_=xown[r0:r0 + 128, :])
                xt = pool.tile([128, KC1, 128], bf16, tag="xt")
                for k in range(KC1):
                    pt = ppt.tile([128, 128], bf16, tag="pt")
                    nc.tensor.transpose(
                        out=pt, in_=xr[:, 128 * k:128 * (k + 1)],
                        identity=ident)
                    nc.vector.tensor_copy(xt[:, k, :], pt)
                pal = pps.tile([128, 2 * H], f32, tag="pal")
                for k in range(KC1):
                    nc.tensor.matmul(pal, xt[:, k, :],
                                     ascat_sb[:, k, :2 * H],
                                     start=(k == 0), stop=(k == KC1 - 1))
                al_sb = pool.tile([128, 32], f32, tag="al")
                nc.vector.tensor_copy(al_sb[:, :2 * H], pal)
                nc.vector.memset(al_sb[:, 2 * H:], 0.0)
                nc.sync.dma_start(out=alsl[r0:r0 + 128, :], in_=al_sb)
        nc.gpsimd.collective_compute(
            "AllGather", mybir.AluOpType.bypass,
            replica_groups=[list(range(NC_))],
            ins=[alsl], outs=[alsp])

        # ---------------- Phase B: GAT blocks -> hsl (kc-slab-major) ---------
        with tc.tile_pool(name="pb_w", bufs=1) as pw, \
             tc.tile_pool(name="pb_sb", bufs=2) as pool, \
             tc.tile_pool(name="pb_sm", bufs=3) as psm, \
             tc.tile_pool(name="pb_m", bufs=EC + 2) as pm, \
             tc.tile_pool(name="pb_ps", bufs=2, space="PSUM") as pps, \
             tc.tile_pool(name="pb_ph", bufs=1, space="PSUM") as pph, \
             tc.tile_pool(name="pb_pz", bufs=2, space="PSUM") as ppz:
            wgat_sb = pw.tile([128, KC1, HF], bf16)
            for k in range(KC1):
                nc.sync.dma_start(out=wgat_sb[:, k, :], in_=wgat[k])

            for b in range(NBLK):
                xe = pool.tile([128, EC, FP], bf16, tag="xe")
                als = pool.tile([128, EC, 32], f32, tag="als")
                ald = pool.tile([128, EC, 32], f32, tag="ald")
                for e in range(EC):
                    col = b * EC + e
                    nc.gpsimd.indirect_dma_start(
                        out=xe[:, e, :], out_offset=None, in_=xb,
                        in_offset=bass.IndirectOffsetOnAxis(
                            ap=esrcx_sb[:, col:col + 1], axis=0))
                    nc.gpsimd.indirect_dma_start(
                        out=als[:, e, :], out_offset=None, in_=alsp,
                        in_offset=bass.IndirectOffsetOnAxis(
                            ap=esrch_sb[:, col:col + 1], axis=0))
                    nc.gpsimd.indirect_dma_start(
                        out=ald[:, e, :], out_offset=None, in_=alsp,
                        in_offset=bass.IndirectOffsetOnAxis(
                            ap=eldst_sb[:, col:col + 1], axis=0))

                masks = []
                exb = psm.tile([128, EC, H], bf16, tag="exb")
                for e in range(EC):
                    col = b * EC + e
                    msk = pm.tile([128, 128], bf16, tag="msk")
                    nc.vector.tensor_tensor(
                        out=msk,
                        in0=dlocc_sb[:, col:col + 1].to_broadcast([128, 128]),
                        in1=iota_f, op=mybir.AluOpType.is_equal)
                    masks.append(msk)
                    # logits -> exp (leaky_relu slope 0.2)
                    lg = psm.tile([128, H], f32, tag="lg")
                    nc.vector.tensor_tensor(out=lg, in0=als[:, e, :H],
                                            in1=ald[:, e, H:2 * H],
                                            op=mybir.AluOpType.add)
                    lg2 = psm.tile([128, H], f32, tag="lg2")
                    nc.vector.tensor_scalar_mul(lg2, lg, 0.2)
                    nc.vector.tensor_tensor(out=lg, in0=lg, in1=lg2,
                                            op=mybir.AluOpType.max)
                    nc.scalar.activation(out=exb[:, e, :], in_=lg,
                                         func=mybir.ActivationFunctionType.Exp)

                # z[d,h] = sum_e mask[e,d] * ex[e,h]
                pz = ppz.tile([128, H], f32, tag="pz")
                for e in range(EC):
                    nc.tensor.matmul(pz, masks[e], exb[:, e, :],
                                     start=(e == 0), stop=(e == EC - 1))
                zf = psm.tile([128, H], f32, tag="zf")
                nc.scalar.activation(out=zf, in_=pz,
                                     func=mybir.ActivationFunctionType.Copy,
                                     bias=1e-30)
                zinv = psm.tile([128, H], f32, tag="zinv")
                nc.vector.reciprocal(zinv, zf)

                # M_e[:, h, :] = mask_e * ex[e, h]  (vector/scalar split)
                aggT = pool.tile([128, KC1, H, 128], bf16, tag="aggT")
                Ms = []
                for e in range(EC):
                    Me = pm.tile([128, H, 128], bf16, tag="Me")
                    for h in range(H):
                        nc.vector.tensor_tensor(
                            out=Me[:, h, :], in0=masks[e],
                            in1=exb[:, e, h:h + 1].to_broadcast([128, 128]),
                            op=mybir.AluOpType.mult)
                    Ms.append(Me)
                # aggT[f, (h d)] += xe.T @ M
                for k in range(KC1):
                    for half in range(2):
                        pa = pps.tile([128, 5 * 128], f32, tag="pa")
                        h0 = 5 * half
                        for e in range(EC):
                            lhs = xe[:, e, 128 * k:128 * (k + 1)]
                            nc.tensor.matmul(pa[:, 0:512], lhs,
                                             Ms[e][:, h0:h0 + 4, :],
                                             start=(e == 0), stop=(e == EC - 1))
                            nc.tensor.matmul(pa[:, 512:640], lhs,
                                             Ms[e][:, h0 + 4:h0 + 5, :],
                                             start=(e == 0), stop=(e == EC - 1))
                        nc.vector.tensor_copy(aggT[:, k, h0:h0 + 5, :], pa)

                # transform per head + fused 1/z scale + relu
                h1 = pool.tile([128, HFP], bf16, tag="h1")
                for h in range(H):
                    ph = pph.tile([128, F], f32, tag="ph")
                    for k in range(KC1):
                        lhs = aggT[:, k, h, :]
                        nc.tensor.matmul(ph[:, 0:512], lhs,
                                         wgat_sb[:, k, F * h:F * h + 512],
                                         start=(k == 0), stop=(k == KC1 - 1))
                        nc.tensor.matmul(ph[:, 512:F], lhs,
                                         wgat_sb[:, k, F * h + 512:F * (h + 1)],
                                         start=(k == 0), stop=(k == KC1 - 1))
                    nc.scalar.activation(out=h1[:, F * h:F * (h + 1)], in_=ph,
                                         func=mybir.ActivationFunctionType.Relu,
                                         scale=zinv[:, h:h + 1])
                nc.vector.memset(h1[:, HF:HFP], 0.0)
                for kc in range(KC2):
                    nc.sync.dma_start(
                        out=hsl[kc, 128 * b:128 * (b + 1), :],
                        in_=h1[:, 128 * kc:128 * (kc + 1)])

        # ---------------- big resident tiles for T/E/D ----------------
        with tc.tile_pool(name="pe_big", bufs=1) as pbig:
            hTs = [pbig.tile([128, RPC], bf16, name=f"hT{kc}")
                   for kc in range(KC2)]
            gT = pbig.tile([128, 2 * KC2, GPC], bf16)

            # --- Phase E + segmented AllGather + Phase D, interleaved ------
            # PE queue order per segment s:
            #   [E fp0..fp2][E fp3][D(s-1) b0][E fp4][D b1] ... [E fp8][D b5]
            #   [D b6][D b7][readout(s-1) + MLP acc][AllGather(s)]
            # D matmuls trail their gathers by ~3 fp chunks so the AllGather
            # latency never head-of-line-blocks the in-order PE queue; wsl
            # weight loads are issued before each fp's transposes so they are
            # never stuck behind them on the sync DMA queue.
            with tc.tile_pool(name="pe_w", bufs=2) as pwp, \
                 tc.tile_pool(name="pe_sb", bufs=2) as pe, \
                 tc.tile_pool(name="pe_xt", bufs=2) as pxt, \
                 tc.tile_pool(name="pe_ps", bufs=2, space="PSUM") as pps3, \
                 tc.tile_pool(name="pd_he", bufs=2) as phe, \
                 tc.tile_pool(name="pd_m", bufs=2) as pmsk, \
                 tc.tile_pool(name="pd_h2", bufs=1) as ph2pool, \
                 tc.tile_pool(name="pd_sb", bufs=2) as pd, \
                 tc.tile_pool(name="pd_ps", bufs=2, space="PSUM") as ppd, \
                 tc.tile_pool(name="pf_w", bufs=4) as pw1, \
                 tc.tile_pool(name="pf_p1", bufs=1, space="PSUM") as pp1:
                p1acc = pp1.tile([32, 512], f32, name="p1acc")
                h2as = {}
                dstate = {}
                wsls = {}

                def load_wsl(fp):
                    if fp < KC2:
                        wsl = pwp.tile([128, KC2, 128], bf16, tag="wsl")
                        nc.sync.dma_start(out=wsl, in_=wgcn[fp])
                        wsls[fp] = wsl

                load_wsl(0)
                load_wsl(1)
                # Phase T: hsl kc-slabs -> hT (DMA transposes); E chases these
                for kc in range(KC2):
                    nc.sync.dma_start_transpose(out=hTs[kc], in_=hsl[kc])

                def e_fp(s, fpi):
                    fp = SEGF * s + fpi
                    wsl = wsls.pop(fp)
                    ph2 = pps3.tile([128, RPC], f32, tag="ph2")
                    for kc in range(KC2):
                        nc.tensor.matmul(ph2[:, 0:512], wsl[:, kc, :],
                                         hTs[kc][:, 0:512],
                                         start=(kc == 0), stop=(kc == KC2 - 1))
                        nc.tensor.matmul(ph2[:, 512:RPC], wsl[:, kc, :],
                                         hTs[kc][:, 512:RPC],
                                         start=(kc == 0), stop=(kc == KC2 - 1))
                    # prefetch two chunks ahead (reuses this fp's pool slot;
                    # issued before the transposes to dodge queue blocking)
                    load_wsl(fp + 2)
                    xwf = pe.tile([128, RPC], bf16, tag="xwf")
                    nc.vector.tensor_copy(xwf, ph2)
                    for nb in range(RPC // 128):
                        xwt = pxt.tile([128, 128], bf16, tag="xwt")
                        nc.sync.dma_start_transpose(
                            out=xwt, in_=xwf[:, 128 * nb:128 * (nb + 1)])
                        nc.sync.dma_start(
                            out=xwsl[s][128 * nb:128 * (nb + 1),
                                        128 * fpi:128 * (fpi + 1)],
                            in_=xwt)

                def d_gathers(s, b):
                    """Prefetch block b's edge rows + masks (no PE work)."""
                    hes, m2s = [], []
                    for e in range(EC):
                        col = b * EC + e
                        he = phe.tile([128, SEGW], bf16, tag=f"he{e}")
                        nc.gpsimd.indirect_dma_start(
                            out=he, out_offset=None, in_=xwpad[s],
                            in_offset=bass.IndirectOffsetOnAxis(
                                ap=esrch_sb[:, col:col + 1], axis=0))
                        hes.append(he)
                        msk = pmsk.tile([128, 128], bf16, tag=f"mk{e}")
                        nc.vector.tensor_tensor(
                            out=msk,
                            in0=dlocc_sb[:, col:col + 1].to_broadcast(
                                [128, 128]),
                            in1=iota_f, op=mybir.AluOpType.is_equal)
                        nc.vector.tensor_tensor(
                            out=msk, in0=msk,
                            in1=normc_sb[:, col:col + 1].to_broadcast(
                                [128, 128]),
                            op=mybir.AluOpType.mult)
                        m2s.append(msk)
                    dstate[(s, b)] = (hes, m2s)

                def d_block(s, b):
                    """Aggregation matmuls + relu drains for block b."""
                    hes, m2s = dstate.pop((s, b))
                    h2a = h2as[s]
                    for kci in range(SEGF):
                        p2 = ppd.tile([128, 128], f32, tag="p2")
                        for e in range(EC):
                            nc.tensor.matmul(
                                p2, hes[e][:, 128 * kci:128 * (kci + 1)],
                                m2s[e],
                                start=(e == 0), stop=(e == EC - 1))
                        nc.scalar.activation(
                            out=h2a[:, kci, 128 * b:128 * (b + 1)], in_=p2,
                            func=mybir.ActivationFunctionType.Relu)

                def d_readout(s):
                    """Graph max/mean per kc + fold in the W1 MLP matmuls."""
                    h2a = h2as.pop(s)
                    for kci in range(SEGF):
                        kc = SEGF * s + kci
                        h2r = h2a[:, kci, :NPC].rearrange(
                            "p (g n) -> p g n", n=NPG)
                        gmax = pd.tile([128, GPC], f32, tag="gmax")
                        nc.vector.tensor_reduce(out=gmax, in_=h2r,
                                                axis=mybir.AxisListType.X,
                                                op=mybir.AluOpType.max)
                        gsum = pd.tile([128, GPC], f32, tag="gsum")
                        nc.vector.tensor_reduce(out=gsum, in_=h2r,
                                                axis=mybir.AxisListType.X,
                                                op=mybir.AluOpType.add)
                        nc.vector.tensor_copy(gT[:, kc, :], gmax)
                        nc.scalar.activation(
                            out=gT[:, KC2 + kc, :], in_=gsum,
                            func=mybir.ActivationFunctionType.Copy,
                            scale=1.0 / NPG)
                        for part, gk in ((0, kc), (1, KC2 + kc)):
                            w1sl = pw1.tile([128, 512], bf16, tag="w1sl")
                            nc.sync.dma_start(out=w1sl, in_=w1t[gk])
                            nc.tensor.matmul(
                                p1acc, gT[:, gk, :], w1sl,
                                start=(kc == 0 and part == 0),
                                stop=(kc == KC2 - 1 and part == 1))

                for s in range(NSEG):
                    if s > 0:
                        h2as[s - 1] = ph2pool.tile([128, SEGF, RPC], bf16,
                                                   tag="h2a", name="h2a")
                    for fpi in range(SEGF):
                        e_fp(s, fpi)
                        if s > 0:
                            if fpi >= 3:
                                d_block(s - 1, fpi - 3)
                            if 1 <= fpi:
                                d_gathers(s - 1, fpi - 1)
                    if s > 0:
                        for b in (6, 7):
                            d_block(s - 1, b)
                        d_readout(s - 1)
                    nc.gpsimd.collective_compute(
                        "AllGather", mybir.AluOpType.bypass,
                        replica_groups=[list(range(NC_))],
                        ins=[xwsl[s]], outs=[xwpad[s]])
                # last segment's aggregation + readout (tail)
                h2as[NSEG - 1] = ph2pool.tile([128, SEGF, RPC], bf16,
                                              tag="h2a", name="h2a")
                for b in range(NBLK):
                    if b >= 2:
                        d_block(NSEG - 1, b - 2)
                    d_gathers(NSEG - 1, b)
                for b in (6, 7):
                    d_block(NSEG - 1, b)
                d_readout(NSEG - 1)

                # ---------------- MLP tail ----------------
                with tc.tile_pool(name="pf_sb", bufs=2) as pf:
                    o1g = pf.tile([32, 512], bf16, tag="o1g")
                    nc.scalar.activation(
                        out=o1g, in_=p1acc,
                        func=mybir.ActivationFunctionType.Relu)
                    o1 = pf.tile([128, 4, 32], bf16, tag="o1")
                    for mc in range(4):
                        pt1 = ppd.tile([128, 32], bf16, tag="p2")
                        nc.tensor.transpose(
                            out=pt1, in_=o1g[:, 128 * mc:128 * (mc + 1)],
                            identity=ident[:32, :32])
                        nc.vector.tensor_copy(o1[:, mc, :], pt1)
                    w2sb = pf.tile([128, 4, 128], bf16, tag="w2sb")
                    nc.sync.dma_start(out=w2sb,
                                      in_=w2t.rearrange("c p f -> p c f"))
                    p2t = ppd.tile([128, 32], f32, tag="p2")
                    for kc in range(4):
                        nc.tensor.matmul(p2t, w2sb[:, kc, :], o1[:, kc, :],
                                         start=(kc == 0), stop=(kc == 3))
                    o2 = pf.tile([128, 32], bf16, tag="o2")
                    nc.vector.tensor_copy(o2, p2t)
                    w3sb = pf.tile([128, 64], bf16, tag="w3sb")
                    nc.sync.dma_start(out=w3sb, in_=w3t)
                    p3t = ppd.tile([64, 32], f32, tag="p2")
                    nc.tensor.matmul(p3t, w3sb, o2, start=True, stop=True)
                    o3 = pf.tile([64, 32], f32, tag="o3")
                    nc.vector.tensor_copy(o3, p3t)
                    nc.sync.dma_start(out=outg, in_=o3)

    nc.compile()
    return nc


_NC_CACHE = None


def get_nc():
    global _NC_CACHE
    if _NC_CACHE is None:
        _NC_CACHE = build_nc()
    return _NC_CACHE


def make_in_maps(inputs):
    shared, per_core = host_prep(inputs)
    return [dict(shared, **pc) for pc in per_core]


def kernel(**inputs):
    from concourse.bass_utils import run_bass_kernel_spmd
    nc = get_nc()
    in_maps = make_in_maps(inputs)
    res = run_bass_kernel_spmd(nc, in_maps, core_ids=list(range(NC_)))
    out = np.zeros((G, 64), np.float32)
    for c in range(NC_):
        out[GPC * c:GPC * (c + 1), :] = res.results[c]["outg"].T
    return out


if __name__ == "__main__":
    d = np.load("/root/problem/inputs.npz")
    inputs = {k: d[k] for k in d.files}
    out = kernel(**inputs)
    print("out", out.shape, out.dtype, out[:2, :4])


# revision 27
# speedup vs baseline: 1.3004x; 1.1350x over previous
"""Trainium2 Bass kernel for nn_GAT_GCN (GAT -> GCN -> readout -> MLP), 8-core SPMD.

v2: inverted GCN (transform-then-aggregate) so the big AllGather pipelines
under the 6800x6800 GCN matmul instead of sitting exposed on the critical
path.

Sharding: 1024 rows per core: 992 graph-aligned nodes (32 graphs x 31) plus
32 orphan rows (the 64 readout-dropped nodes 7936..7999 split across cores
6 and 7). Edges are owned by their dst node; 8 dst blocks of 128 per core.

Pipeline per core:
- A: al = x @ [As|Ad] for OWN nodes only; AllGather -> alsp_pad[8192,32].
- B: GAT per dst block (one-hot mask matmul aggregation, softmax 1/z folded
  into the PSUM-drain activation scale); h written to DRAM kc-slab-major.
- T: h -> hT[128,54,1024] (feat-major, SBUF resident) via DMA transposes.
- E: xw = h @ W_gcn on own nodes (fp-chunk loop, wgcn streamed); xw rows
  re-transposed to node-major and AllGathered in 6 column segments, each
  issued as soon as its 9 fp chunks finish -> comm hides under E compute.
- D: GCN aggregation per segment via norm-weighted one-hot masks (gathering
  xw rows per edge), fused relu + graph max/mean readout; interleaved with
  E so segment s aggregates while segment s+1 transforms.
- MLP on [64,32] transposed output; host concatenates.
"""
import sys
import numpy as np
import ml_dtypes

sys.path.insert(0, "/opt/trn_rl_repo")

from contextlib import ExitStack  # noqa: E402

import concourse.bass as bass  # noqa: E402
import concourse.tile as tile  # noqa: E402
from concourse import bacc, mybir  # noqa: E402

N, E, G = 8000, 32000, 256
F, H = 680, 10
HF = F * H                    # 6800
NC_ = 8                       # cores
NPC = 992                     # readout nodes per core (32 graphs x 31)
RPC = 1024                    # rows per core (992 + 32 orphan slots)
NBLK = 8                      # dst blocks per core (128 each)
EB = 768                      # padded edges per block
EC = EB // 128                # 6 edge chunks
FP = 768                      # padded F
KC1 = FP // 128               # 6
HFP = 6912                    # padded HF
KC2 = HFP // 128              # 54
NSEG = 6                      # xw AllGather segments
SEGF = KC2 // NSEG            # 9 fp chunks per segment
SEGW = SEGF * 128             # 1152 cols per segment
GPC = 32                      # graphs per core
NPG = 31                      # nodes per graph

f32 = mybir.dt.float32
bf16 = mybir.dt.bfloat16
i32 = mybir.dt.int32
bfnp = ml_dtypes.bfloat16


# ----------------------------------------------------------------------------
# Host-side prep: sharding, padding, weight tiling
# ----------------------------------------------------------------------------

def node_owner_local(node):
    """node -> (owner core, local row) for the 992x8 + 32/32 orphan layout."""
    node = np.asarray(node)
    owner = np.where(node < 7936, node // NPC,
                     np.where(node < 7968, 6, 7))
    local = np.where(node < 7936, node - NPC * (node // NPC),
                     np.where(node < 7968, NPC + node - 7936,
                              NPC + node - 7968))
    return owner, local


def host_prep(inputs):
    x = np.asarray(inputs["x"], np.float32)
    edge_index = np.asarray(inputs["edge_index"])
    W_gat = np.asarray(inputs["W_gat"], np.float32)
    a_src = np.asarray(inputs["a_src"], np.float32)
    a_dst = np.asarray(inputs["a_dst"], np.float32)
    W_gcn = np.asarray(inputs["W_gcn"], np.float32)
    W1 = np.asarray(inputs["W1"], np.float32)
    W2 = np.asarray(inputs["W2"], np.float32)
    W3 = np.asarray(inputs["W3"], np.float32)
    for bname in ("b_gat", "b_gcn", "b1", "b2", "b3"):
        assert np.all(np.asarray(inputs[bname]) == 0), f"nonzero {bname}"

    src = np.concatenate([edge_index[0], np.arange(N)]).astype(np.int64)
    dst = np.concatenate([edge_index[1], np.arange(N)]).astype(np.int64)
    deg = np.bincount(dst, minlength=N).astype(np.float64)
    dinv = 1.0 / np.sqrt(deg)
    norm = (dinv[src] * dinv[dst]).astype(np.float32)

    owner_n, local_n = node_owner_local(np.arange(N))
    hpos = RPC * owner_n + local_n                       # node -> global row

    xb = np.zeros((N, FP), bfnp)
    xb[:, :F] = x.astype(bfnp)

    As = np.stack([W_gat[:, h * F:(h + 1) * F] @ a_src[h] for h in range(H)], 1)
    Ad = np.stack([W_gat[:, h * F:(h + 1) * F] @ a_dst[h] for h in range(H)], 1)
    ascat = np.zeros((FP, 64), bfnp)
    ascat[:F, :H] = As.astype(bfnp)
    ascat[:F, H:2 * H] = Ad.astype(bfnp)

    wgat = np.zeros((KC1, 128, HF), bfnp)
    wgat.reshape(FP, HF)[:F] = W_gat.astype(bfnp)

    wpad = np.zeros((HFP, HFP), np.float32)
    wpad[:HF, :HF] = W_gcn
    # [fp, i(k row in chunk), kc, j] -> per-partition contiguous DMA slabs
    wgcn = np.ascontiguousarray(
        wpad.reshape(KC2, 128, KC2, 128).transpose(2, 1, 0, 3)).astype(bfnp)

    # gT k-order: 54 gmp chunks (rows [0,HF) + 16 pad) then 54 gap chunks
    w1t = np.zeros((2 * KC2, 128, 512), bfnp)
    w1t.reshape(2 * HFP, 512)[:HF] = W1[:HF].astype(bfnp)
    w1t.reshape(2 * HFP, 512)[HFP:HFP + HF] = W1[HF:].astype(bfnp)
    w2t = np.ascontiguousarray(W2.reshape(4, 128, 128)).astype(bfnp)
    w3t = np.ascontiguousarray(W3).astype(bfnp)              # [128, 64]

    shared = dict(xb=xb, ascat=ascat, wgat=wgat, wgcn=wgcn,
                  w1t=w1t, w2t=w2t, w3t=w3t)

    # per-core own-node x slices (hpos-local row order)
    own_nodes = [np.where(owner_n == c)[0][np.argsort(local_n[owner_n == c])]
                 for c in range(NC_)]

    per_core = []
    for c in range(NC_):
        nodes_c = own_nodes[c]
        xown = np.zeros((RPC, FP), bfnp)
        xown[local_n[nodes_c]] = xb[nodes_c]

        esrcx = np.zeros((NBLK, EC, 128), np.int32)
        esrch = np.zeros((NBLK, EC, 128), np.int32)
        eldst = np.zeros((NBLK, EC, 128), np.int32)
        dlocc = np.full((NBLK, EC, 128), -1.0, np.float32)
        normc = np.zeros((NBLK, EC, 128), np.float32)
        em = (owner_n[dst] == c)
        es, ed, en = src[em], dst[em], norm[em]
        loc = local_n[ed]
        for b in range(NBLK):
            bm = (loc >= 128 * b) & (loc < 128 * (b + 1))
            cnt = int(bm.sum())
            assert cnt <= EB, (c, b, cnt)
            fs = np.zeros(EB, np.int64)
            fd = np.zeros(EB, np.int64)
            fl = np.full(EB, -1.0, np.float32)
            fn = np.zeros(EB, np.float32)
            fs[:cnt] = es[bm]
            fd[:cnt] = ed[bm]
            fl[:cnt] = (loc[bm] - 128 * b).astype(np.float32)
            fn[:cnt] = en[bm]
            esrcx[b] = fs.reshape(EC, 128)
            esrch[b] = hpos[fs].reshape(EC, 128)
            eldst[b] = hpos[fd].reshape(EC, 128)
            dlocc[b] = fl.reshape(EC, 128)
            normc[b] = fn.reshape(EC, 128)
        pc = dict(
            xown=xown,
            esrcx=np.ascontiguousarray(esrcx.reshape(NBLK * EC, 128).T),
            esrch=np.ascontiguousarray(esrch.reshape(NBLK * EC, 128).T),
            eldst=np.ascontiguousarray(eldst.reshape(NBLK * EC, 128).T),
            dlocc=np.ascontiguousarray(dlocc.reshape(NBLK * EC, 128).T),
            normc=np.ascontiguousarray(
                normc.reshape(NBLK * EC, 128).T.astype(bfnp)),
        )
        per_core.append(pc)
    return shared, per_core


# ----------------------------------------------------------------------------
# Device program (one SPMD Bass program; all per-core variation is via data)
# ----------------------------------------------------------------------------

def build_nc():
    nc = bacc.Bacc("TRN2", target_bir_lowering=False, debug=False,
                   num_devices=NC_)
    xb = nc.dram_tensor("xb", [N, FP], bf16, kind="ExternalInput").ap()
    xown = nc.dram_tensor("xown", [RPC, FP], bf16, kind="ExternalInput").ap()
    ascat = nc.dram_tensor("ascat", [FP, 64], bf16, kind="ExternalInput").ap()
    wgat = nc.dram_tensor("wgat", [KC1, 128, HF], bf16, kind="ExternalInput").ap()
    wgcn = nc.dram_tensor("wgcn", [KC2, 128, KC2, 128], bf16,
                          kind="ExternalInput").ap()
    w1t = nc.dram_tensor("w1t", [2 * KC2, 128, 512], bf16,
                         kind="ExternalInput").ap()
    w2t = nc.dram_tensor("w2t", [4, 128, 128], bf16, kind="ExternalInput").ap()
    w3t = nc.dram_tensor("w3t", [128, 64], bf16, kind="ExternalInput").ap()
    esrcx = nc.dram_tensor("esrcx", [128, NBLK * EC], i32,
                           kind="ExternalInput").ap()
    esrch = nc.dram_tensor("esrch", [128, NBLK * EC], i32,
                           kind="ExternalInput").ap()
    eldst = nc.dram_tensor("eldst", [128, NBLK * EC], i32,
                           kind="ExternalInput").ap()
    dlocc = nc.dram_tensor("dlocc", [128, NBLK * EC], f32,
                           kind="ExternalInput").ap()
    normc = nc.dram_tensor("normc", [128, NBLK * EC], bf16,
                           kind="ExternalInput").ap()
    outg = nc.dram_tensor("outg", [64, 32], f32, kind="ExternalOutput").ap()

    with tile.TileContext(nc) as tc, ExitStack() as ctx:
        dram = ctx.enter_context(tc.tile_pool(name="dram", bufs=1, space="DRAM"))
        alsl = dram.tile([RPC, 32], f32, name="alsl")
        alsp = dram.tile([NC_ * RPC, 32], f32, name="alsp", addr_space="Shared")
        hsl = dram.tile([KC2, RPC, 128], bf16, name="hsl")
        xwsl = [dram.tile([RPC, SEGW], bf16, name=f"xwsl{s}")
                for s in range(NSEG)]
        xwfm = dram.tile([KC2, 128, RPC], bf16, name="xwfm")
        xwpad = [dram.tile([NC_ * RPC, SEGW], bf16, name=f"xwpad{s}",
                           addr_space="Shared") for s in range(NSEG)]
        singles = ctx.enter_context(tc.tile_pool(name="singles", bufs=1))

        iota_i = singles.tile([128, 128], i32)
        nc.gpsimd.iota(iota_i, pattern=[[1, 128]], base=0, channel_multiplier=0)
        iota_f = singles.tile([128, 128], f32)
        nc.vector.tensor_copy(iota_f, iota_i)

        ascat_sb = singles.tile([128, KC1, 64], bf16)
        nc.sync.dma_start(out=ascat_sb,
                          in_=ascat.rearrange("(c p) d -> p c d", p=128))
        esrcx_sb = singles.tile([128, NBLK * EC], i32)
        nc.sync.dma_start(out=esrcx_sb, in_=esrcx)
        esrch_sb = singles.tile([128, NBLK * EC], i32)
        nc.sync.dma_start(out=esrch_sb, in_=esrch)
        eldst_sb = singles.tile([128, NBLK * EC], i32)
        nc.sync.dma_start(out=eldst_sb, in_=eldst)
        dlocc_sb = singles.tile([128, NBLK * EC], f32)
        nc.sync.dma_start(out=dlocc_sb, in_=dlocc)
        normc_sb = singles.tile([128, NBLK * EC], bf16)
        nc.sync.dma_start(out=normc_sb, in_=normc)

        from concourse.masks import make_identity
        ident = singles.tile([128, 128], bf16, name="ident")
        make_identity(nc, ident)

        # ---------------- Phase A: al = x_own @ [As|Ad] -> AllGather ----------
        with tc.tile_pool(name="pa_sb", bufs=3) as pool, \
             tc.tile_pool(name="pa_ps", bufs=2, space="PSUM") as pps, \
             tc.tile_pool(name="pa_pt", bufs=4, space="PSUM") as ppt:
            for i in range(RPC // 128):
                r0 = 128 * i
                xr = pool.tile([128, FP], bf16, tag="xr")
                nc.sync.dma_start(out=xr, in_=xown[r0:r0 + 128, :])
                xt = pool.tile([128, KC1, 128], bf16, tag="xt")
                for k in range(KC1):
                    pt = ppt.tile([128, 128], bf16, tag="pt")
                    nc.tensor.transpose(
                        out=pt, in_=xr[:, 128 * k:128 * (k + 1)],
                        identity=ident)
                    nc.vector.tensor_copy(xt[:, k, :], pt)
                pal = pps.tile([128, 2 * H], f32, tag="pal")
                for k in range(KC1):
                    nc.tensor.matmul(pal, xt[:, k, :],
                                     ascat_sb[:, k, :2 * H],
                                     start=(k == 0), stop=(k == KC1 - 1))
                al_sb = pool.tile([128, 32], f32, tag="al")
                nc.vector.tensor_copy(al_sb[:, :2 * H], pal)
                nc.vector.memset(al_sb[:, 2 * H:], 0.0)
                nc.sync.dma_start(out=alsl[r0:r0 + 128, :], in_=al_sb)
        nc.gpsimd.collective_compute(
            "AllGather", mybir.AluOpType.bypass,
            replica_groups=[list(range(NC_))],
            ins=[alsl], outs=[alsp])

        # ---------------- Phase B: GAT blocks -> hsl (kc-slab-major) ---------
        with tc.tile_pool(name="pb_w", bufs=1) as pw, \
             tc.tile_pool(name="pb_sb", bufs=2) as pool, \
             tc.tile_pool(name="pb_sm", bufs=3) as psm, \
             tc.tile_pool(name="pb_m", bufs=EC + 2) as pm, \
             tc.tile_pool(name="pb_ps", bufs=2, space="PSUM") as pps, \
             tc.tile_pool(name="pb_ph", bufs=1, space="PSUM") as pph, \
             tc.tile_pool(name="pb_pz", bufs=2, space="PSUM") as ppz:
            wgat_sb = pw.tile([128, KC1, HF], bf16)
            for k in range(KC1):
                nc.sync.dma_start(out=wgat_sb[:, k, :], in_=wgat[k])

            for b in range(NBLK):
                xe = pool.tile([128, EC, FP], bf16, tag="xe")
                als = pool.tile([128, EC, 32], f32, tag="als")
                ald = pool.tile([128, EC, 32], f32, tag="ald")
                for e in range(EC):
                    col = b * EC + e
                    nc.gpsimd.indirect_dma_start(
                        out=xe[:, e, :], out_offset=None, in_=xb,
                        in_offset=bass.IndirectOffsetOnAxis(
                            ap=esrcx_sb[:, col:col + 1], axis=0))
                    nc.gpsimd.indirect_dma_start(
                        out=als[:, e, :], out_offset=None, in_=alsp,
                        in_offset=bass.IndirectOffsetOnAxis(
                            ap=esrch_sb[:, col:col + 1], axis=0))
                    nc.gpsimd.indirect_dma_start(
                        out=ald[:, e, :], out_offset=None, in_=alsp,
                        in_offset=bass.IndirectOffsetOnAxis(
                            ap=eldst_sb[:, col:col + 1], axis=0))

                masks = []
                exb = psm.tile([128, EC, H], bf16, tag="exb")
                for e in range(EC):
                    col = b * EC + e
                    msk = pm.tile([128, 128], bf16, tag="msk")
                    nc.vector.tensor_tensor(
                        out=msk,
                        in0=dlocc_sb[:, col:col + 1].to_broadcast([128, 128]),
                        in1=iota_f, op=mybir.AluOpType.is_equal)
                    masks.append(msk)
                    # logits -> exp (leaky_relu slope 0.2)
                    lg = psm.tile([128, H], f32, tag="lg")
                    nc.vector.tensor_tensor(out=lg, in0=als[:, e, :H],
                                            in1=ald[:, e, H:2 * H],
                                            op=mybir.AluOpType.add)
                    lg2 = psm.tile([128, H], f32, tag="lg2")
                    nc.vector.tensor_scalar_mul(lg2, lg, 0.2)
                    nc.vector.tensor_tensor(out=lg, in0=lg, in1=lg2,
                                            op=mybir.AluOpType.max)
                    nc.scalar.activation(out=exb[:, e, :], in_=lg,
                                         func=mybir.ActivationFunctionType.Exp)

                # z[d,h] = sum_e mask[e,d] * ex[e,h]
                pz = ppz.tile([128, H], f32, tag="pz")
                for e in range(EC):
                    nc.tensor.matmul(pz, masks[e], exb[:, e, :],
                                     start=(e == 0), stop=(e == EC - 1))
                zf = psm.tile([128, H], f32, tag="zf")
                nc.scalar.activation(out=zf, in_=pz,
                                     func=mybir.ActivationFunctionType.Copy,
                                     bias=1e-30)
                zinv = psm.tile([128, H], f32, tag="zinv")
                nc.vector.reciprocal(zinv, zf)

                # M_e[:, h, :] = mask_e * ex[e, h]  (vector/scalar split)
                aggT = pool.tile([128, KC1, H, 128], bf16, tag="aggT")
                Ms = []
                for e in range(EC):
                    Me = pm.tile([128, H, 128], bf16, tag="Me")
                    for h in range(H):
                        nc.vector.tensor_tensor(
                            out=Me[:, h, :], in0=masks[e],
                            in1=exb[:, e, h:h + 1].to_broadcast([128, 128]),
                            op=mybir.AluOpType.mult)
                    Ms.append(Me)
                # aggT[f, (h d)] += xe.T @ M
                for k in range(KC1):
                    for half in range(2):
                        pa = pps.tile([128, 5 * 128], f32, tag="pa")
                        h0 = 5 * half
                        for e in range(EC):
                            lhs = xe[:, e, 128 * k:128 * (k + 1)]
                            nc.tensor.matmul(pa[:, 0:512], lhs,
                                             Ms[e][:, h0:h0 + 4, :],
                                             start=(e == 0), stop=(e == EC - 1))
                            nc.tensor.matmul(pa[:, 512:640], lhs,
                                             Ms[e][:, h0 + 4:h0 + 5, :],
                                             start=(e == 0), stop=(e == EC - 1))
                        nc.vector.tensor_copy(aggT[:, k, h0:h0 + 5, :], pa)

                # transform per head + fused 1/z scale + relu
                h1 = pool.tile([128, HFP], bf16, tag="h1")
                for h in range(H):
                    ph = pph.tile([128, F], f32, tag="ph")
                    for k in range(KC1):
                        lhs = aggT[:, k, h, :]
                        nc.tensor.matmul(ph[:, 0:512], lhs,
                                         wgat_sb[:, k, F * h:F * h + 512],
                                         start=(k == 0), stop=(k == KC1 - 1))
                        nc.tensor.matmul(ph[:, 512:F], lhs,
                                         wgat_sb[:, k, F * h + 512:F * (h + 1)],
                                         start=(k == 0), stop=(k == KC1 - 1))
                    nc.scalar.activation(out=h1[:, F * h:F * (h + 1)], in_=ph,
                                         func=mybir.ActivationFunctionType.Relu,
                                         scale=zinv[:, h:h + 1])
                nc.vector.memset(h1[:, HF:HFP], 0.0)
                for kc in range(KC2):
                    nc.sync.dma_start(
                        out=hsl[kc, 128 * b:128 * (b + 1), :],
                        in_=h1[:, 128 * kc:128 * (kc + 1)])

        # ---------------- big resident tiles for T/E/D ----------------
        with tc.tile_pool(name="pe_big", bufs=1) as pbig:
            hTs = [pbig.tile([128, RPC], bf16, name=f"hT{kc}")
                   for kc in range(KC2)]
            gT = pbig.tile([128, 2 * KC2, GPC], bf16)

            # --- Phase E + segmented AllGather + Phase D, interleaved ------
            # PE queue order per segment s:
            #   [E fp0..fp2][E fp3][D(s-1) b0][E fp4][D b1] ... [E fp8][D b5]
            #   [D b6][D b7][readout(s-1) + MLP acc][AllGather(s)]
            # D matmuls trail their gathers by ~3 fp chunks so the AllGather
            # latency never head-of-line-blocks the in-order PE queue; wsl
            # weight loads are issued before each fp's transposes so they are
            # never stuck behind them on the sync DMA queue.
            with tc.tile_pool(name="pe_w", bufs=2) as pwp, \
                 tc.tile_pool(name="pe_sb", bufs=2) as pe, \
                 tc.tile_pool(name="pe_xt", bufs=2) as pxt, \
                 tc.tile_pool(name="pe_ps", bufs=2, space="PSUM") as pps3, \
                 tc.tile_pool(name="pd_he", bufs=2) as phe, \
                 tc.tile_pool(name="pd_m", bufs=2) as pmsk, \
                 tc.tile_pool(name="pd_h2", bufs=1) as ph2pool, \
                 tc.tile_pool(name="pd_sb", bufs=2) as pd, \
                 tc.tile_pool(name="pd_ps", bufs=2, space="PSUM") as ppd, \
                 tc.tile_pool(name="pf_w", bufs=2) as pw1, \
                 tc.tile_pool(name="pf_p1", bufs=1, space="PSUM") as pp1:
                p1acc = pp1.tile([32, 512], f32, name="p1acc")
                h2as = {}
                dstate = {}
                wsls = {}

                def load_wsl(fp):
                    if fp < KC2:
                        wsl = pwp.tile([128, KC2, 128], bf16, tag="wsl")
                        nc.sync.dma_start(out=wsl, in_=wgcn[fp])
                        wsls[fp] = wsl

                load_wsl(0)
                load_wsl(1)
                # Phase T: hsl kc-slabs -> hT (DMA transposes); E chases these
                for kc in range(KC2):
                    nc.sync.dma_start_transpose(out=hTs[kc], in_=hsl[kc])

                def e_fp(s, fpi):
                    fp = SEGF * s + fpi
                    wsl = wsls.pop(fp)
                    ph2 = pps3.tile([128, RPC], f32, tag="ph2")
                    for kc in range(KC2):
                        nc.tensor.matmul(ph2[:, 0:512], wsl[:, kc, :],
                                         hTs[kc][:, 0:512],
                                         start=(kc == 0), stop=(kc == KC2 - 1))
                        nc.tensor.matmul(ph2[:, 512:RPC], wsl[:, kc, :],
                                         hTs[kc][:, 512:RPC],
                                         start=(kc == 0), stop=(kc == KC2 - 1))
                    # prefetch two chunks ahead (reuses this fp's pool slot;
                    # issued before the transposes to dodge queue blocking)
                    load_wsl(fp + 2)
                    xwf = pe.tile([128, RPC], bf16, tag="xwf")
                    nc.vector.tensor_copy(xwf, ph2)
                    nc.sync.dma_start(out=xwfm[fp], in_=xwf)

                def xw_transpose(s):
                    """Per node-block: one big DRAM->SBUF XBAR transpose of
                    the segment's 9 fp slabs, then a contiguous write into
                    the AllGather input. (The [1152,128]->[128,1152]
                    direction is fast; per-[128,128] SBUF transposes are not.)"""
                    seg = xwfm[SEGF * s:SEGF * (s + 1)]
                    for nb in range(RPC // 128):
                        xwt = pxt.tile([128, SEGW], bf16, tag="xwt")
                        nc.sync.dma_start_transpose(
                            out=xwt,
                            in_=seg[:, :, 128 * nb:128 * (nb + 1)].rearrange(
                                "a b c -> (a b) c"))
                        nc.sync.dma_start(
                            out=xwsl[s][128 * nb:128 * (nb + 1), :], in_=xwt)

                def d_gathers(s, b):
                    """Prefetch block b's edge rows + masks (no PE work)."""
                    hes, m2s = [], []
                    for e in range(EC):
                        col = b * EC + e
                        he = phe.tile([128, SEGW], bf16, tag=f"he{e}")
                        nc.gpsimd.indirect_dma_start(
                            out=he, out_offset=None, in_=xwpad[s],
                            in_offset=bass.IndirectOffsetOnAxis(
                                ap=esrch_sb[:, col:col + 1], axis=0))
                        hes.append(he)
                        msk = pmsk.tile([128, 128], bf16, tag=f"mk{e}")
                        nc.vector.tensor_tensor(
                            out=msk,
                            in0=dlocc_sb[:, col:col + 1].to_broadcast(
                                [128, 128]),
                            in1=iota_f, op=mybir.AluOpType.is_equal)
                        nc.vector.tensor_tensor(
                            out=msk, in0=msk,
                            in1=normc_sb[:, col:col + 1].to_broadcast(
                                [128, 128]),
                            op=mybir.AluOpType.mult)
                        m2s.append(msk)
                    dstate[(s, b)] = (hes, m2s)

                def d_block(s, b):
                    """Aggregation matmuls + relu drains for block b."""
                    hes, m2s = dstate.pop((s, b))
                    h2a = h2as[s]
                    for kci in range(SEGF):
                        p2 = ppd.tile([128, 128], f32, tag="p2")
                        for e in range(EC):
                            nc.tensor.matmul(
                                p2, hes[e][:, 128 * kci:128 * (kci + 1)],
                                m2s[e],
                                start=(e == 0), stop=(e == EC - 1))
                        nc.scalar.activation(
                            out=h2a[:, kci, 128 * b:128 * (b + 1)], in_=p2,
                            func=mybir.ActivationFunctionType.Relu)

                def d_readout(s):
                    """Graph max/mean per kc + fold in the W1 MLP matmuls."""
                    h2a = h2as.pop(s)
                    for kci in range(SEGF):
                        kc = SEGF * s + kci
                        h2r = h2a[:, kci, :NPC].rearrange(
                            "p (g n) -> p g n", n=NPG)
                        gmax = pd.tile([128, GPC], f32, tag="gmax")
                        nc.vector.tensor_reduce(out=gmax, in_=h2r,
                                                axis=mybir.AxisListType.X,
                                                op=mybir.AluOpType.max)
                        gsum = pd.tile([128, GPC], f32, tag="gsum")
                        nc.vector.tensor_reduce(out=gsum, in_=h2r,
                                                axis=mybir.AxisListType.X,
                                                op=mybir.AluOpType.add)
                        nc.vector.tensor_copy(gT[:, kc, :], gmax)
                        nc.scalar.activation(
                            out=gT[:, KC2 + kc, :], in_=gsum,
                            func=mybir.ActivationFunctionType.Copy,
                            scale=1.0 / NPG)
                        for part, gk in ((0, kc), (1, KC2 + kc)):
                            w1sl = pw1.tile([128, 512], bf16, tag="w1sl")
                            nc.sync.dma_start(out=w1sl, in_=w1t[gk])
                            nc.tensor.matmul(
                                p1acc, gT[:, gk, :], w1sl,
                                start=(kc == 0 and part == 0),
                                stop=(kc == KC2 - 1 and part == 1))

                for s in range(NSEG):
                    if s > 0:
                        h2as[s - 1] = ph2pool.tile([128, SEGF, RPC], bf16,
                                                   tag="h2a", name="h2a")
                    for fpi in range(SEGF):
                        e_fp(s, fpi)
                        if s > 0:
                            if fpi >= 3:
                                d_block(s - 1, fpi - 3)
                            if 1 <= fpi:
                                d_gathers(s - 1, fpi - 1)
                    xw_transpose(s)
                    if s > 0:
                        for b in (6, 7):
                            d_block(s - 1, b)
                        d_readout(s - 1)
                    nc.gpsimd.collective_compute(
                        "AllGather", mybir.AluOpType.bypass,
                        replica_groups=[list(range(NC_))],
                        ins=[xwsl[s]], outs=[xwpad[s]])
                # last segment's aggregation + readout (tail)
                h2as[NSEG - 1] = ph2pool.tile([128, SEGF, RPC], bf16,
                                              tag="h2a", name="h2a")
                for b in range(NBLK):
                    if b >= 2:
                        d_block(NSEG - 1, b - 2)
                    d_gathers(NSEG - 1, b)
                for b in (6, 7):
                    d_block(NSEG - 1, b)
                d_readout(NSEG - 1)

                # ---------------- MLP tail ----------------
                with tc.tile_pool(name="pf_sb", bufs=1) as pf:
                    o1g = pf.tile([32, 512], bf16, tag="o1g")
                    nc.scalar.activation(
                        out=o1g, in_=p1acc,
                        func=mybir.ActivationFunctionType.Relu)
                    o1 = pf.tile([128, 4, 32], bf16, tag="o1")
                    for mc in range(4):
                        pt1 = ppd.tile([128, 32], bf16, tag="p2")
                        nc.tensor.transpose(
                            out=pt1, in_=o1g[:, 128 * mc:128 * (mc + 1)],
                            identity=ident[:32, :32])
                        nc.vector.tensor_copy(o1[:, mc, :], pt1)
                    w2sb = pf.tile([128, 4, 128], bf16, tag="w2sb")
                    nc.sync.dma_start(out=w2sb,
                                      in_=w2t.rearrange("c p f -> p c f"))
                    p2t = ppd.tile([128, 32], f32, tag="p2")
                    for kc in range(4):
                        nc.tensor.matmul(p2t, w2sb[:, kc, :], o1[:, kc, :],
                                         start=(kc == 0), stop=(kc == 3))
                    o2 = pf.tile([128, 32], bf16, tag="o2")
                    nc.vector.tensor_copy(o2, p2t)
                    w3sb = pf.tile([128, 64], bf16, tag="w3sb")
                    nc.sync.dma_start(out=w3sb, in_=w3t)
                    p3t = ppd.tile([64, 32], f32, tag="p2")
                    nc.tensor.matmul(p3t, w3sb, o2, start=True, stop=True)
                    o3 = pf.tile([64, 32], f32, tag="o3")
                    nc.vector.tensor_copy(o3, p3t)
                    nc.sync.dma_start(out=outg, in_=o3)

    nc.compile()
    return nc


_NC_CACHE = None


def get_nc():
    global _NC_CACHE
    if _NC_CACHE is None:
        _NC_CACHE = build_nc()
    return _NC_CACHE


def make_in_maps(inputs):
    shared, per_core = host_prep(inputs)
    return [dict(shared, **pc) for pc in per_core]


def kernel(**inputs):
    from concourse.bass_utils import run_bass_kernel_spmd
    nc = get_nc()
    in_maps = make_in_maps(inputs)
    res = run_bass_kernel_spmd(nc, in_maps, core_ids=list(range(NC_)))
    out = np.zeros((G, 64), np.float32)
    for c in range(NC_):
        out[GPC * c:GPC * (c + 1), :] = res.results[c]["outg"].T
    return out


if __name__ == "__main__":
    d = np.load("/root/problem/inputs.npz")
    inputs = {k: d[k] for k in d.files}
    out = kernel(**inputs)
    print("out", out.shape, out.dtype, out[:2, :4])
